# revision 1
# baseline (speedup 1.0000x reference)
"""Trainium2 Bass kernel for nn_Decoder (latent-grid decoder MLP).

Contract: kernel(**inputs) takes the FULL unsharded inputs (as produced by
setup_inputs()) and returns the FULL [65536, 4] float32 output. Internally the
65536 points are sharded across 8 NeuronCores (pure data parallel); the small
weights are replicated.

Algorithm (mathematically equivalent to the reference):
  - G=2 trilinear interp of a per-sample 2x2x2 grid always lands in cell
    (0,0,0) (indices clip to [0, G-2] = [0,0]), so
    lat_i = sum_m w_m(xyz) * (lat @ A_m), A_m = convT_w[:, :, di, dj, dk].
  - The interp + Fourier features + first MLP layer fold into one matmul:
    u = [w_0*lat, ..., w_7*lat, sin(2 pi ang), cos(2 pi ang)]  (2304 dims),
    h0 = u @ M0 with M0 = [A_stack @ W0_top; W0_sin; W0_cos] (host-folded).
  - LayerNorm mean-subtraction folds into the weights (column centering);
    ln gamma folds in too. The per-sample rstd is deferred via LN's positive
    scale invariance: activations stay unnormalized, and gi2 (squared inverse
    scale) follows gi2' = ssq_w/512 + eps*gi2, applied once at the end.
    Requires all biases and ln_b == 0 (true for this model; a numpy fallback
    covers the general case).
  - ssq_w and the eps*gi2 term are accumulated by the TensorEngine itself
    (weighted-ones stationary operands producing a broadcast [128, S] PSUM
    tile), so no partition reductions or per-sample row ops are needed.
Activations live in [feature, sample] layout; matmuls run as fp32r (full PE
rate at N=512).
"""

import os
import numpy as np

N_CORES = 8
N_TOTAL = 65536
S_CORE = N_TOTAL // N_CORES          # 8192 samples per core
BLK = 512                            # samples per block
N_BLOCKS = S_CORE // BLK             # 16
EPS = 1e-5
N_LAYERS = 8                         # LN+relu layers (layer0 + 7 hidden)


def _precompute(inputs):
    """Host-side weight folding. Returns dict of constant arrays (fp32)."""
    convT_w = np.asarray(inputs["convT_w"], np.float32)
    W0 = np.asarray(inputs["W0"], np.float32)
    Wh = np.asarray(inputs["Wh"], np.float32)
    ln_g = np.asarray(inputs["ln_g"], np.float32)
    gauss = np.asarray(inputs["gauss"], np.float32)
    W_out = np.asarray(inputs["W_out"], np.float32)

    # A_stack[m*256+i, c] = convT_w[i, c, di, dj, dk], m = 4*di + 2*dj + dk
    A_stack = convT_w.transpose(2, 3, 4, 0, 1).reshape(8 * 256, 512)
    M0 = np.concatenate([A_stack @ W0[:512], W0[512:640], W0[640:768]], axis=0)

    def center_scale(W, g):
        Wc = W - W.mean(axis=1, keepdims=True)
        return np.ascontiguousarray(Wc * g[None, :], np.float32)

    W_eff = [center_scale(M0, ln_g[0])] + [
        center_scale(Wh[l], ln_g[l + 1]) for l in range(7)
    ]
    # pack each layer's weights as [128, n_kchunks, 512]
    def pack(W):
        K = W.shape[0]
        kc = K // 128
        return W.reshape(kc, 128, 512).transpose(1, 0, 2).reshape(128, kc * 512)

    w0p = np.ascontiguousarray(pack(W_eff[0]))                       # [128, 18*512]
    whp = np.ascontiguousarray(
        np.concatenate([pack(W) for W in W_eff[1:]], axis=1))        # [128, 28*512]
    # stats lhsT, per layer j and feature chunk mc:
    # tile[k, mc*128 + m] = 1/(512 * g_j[mc*128+k]^2)  (replicated along m)
    sw_cols = []
    for j in (6, 7):
        swv = (1.0 / (512.0 * ln_g[j] ** 2)).astype(np.float32)
        t = np.empty((128, 512), np.float32)
        for mc in range(4):
            t[:, mc * 128:(mc + 1) * 128] = swv[mc * 128:(mc + 1) * 128, None]
        sw_cols.append(t)
    swp = np.ascontiguousarray(np.concatenate(sw_cols, axis=1))      # [128, 2*512]

    return {
        "w0p": w0p,
        "whp": whp,
        "swp": swp,
        "ident": np.eye(128, dtype=np.float32),
        "gaussT": np.ascontiguousarray(gauss.T.astype(np.float32)),  # [3, 128]
        "sel8": np.ascontiguousarray(
            np.kron(np.eye(8, dtype=np.float32), np.ones((1, 128), np.float32))),
        "woutp": np.ascontiguousarray(
            W_out.reshape(4, 128, 4).transpose(1, 0, 2).reshape(128, 16)),
    }


def _general_case_needed(inputs):
    z = lambda a: bool(np.all(np.asarray(a) == 0))
    return not (
        z(inputs["convT_b"]) and z(inputs["b0"]) and z(inputs["bh"])
        and z(inputs["ln_b"]) and z(inputs["b_out"])
        and bool(np.all(np.abs(np.asarray(inputs["ln_g"])) > 1e-3))
    )


def _numpy_fallback(inputs):
    """Reference in numpy (slow; only for inputs outside the fast path)."""
    inp = np.asarray(inputs["input"], np.float32)
    convT_w = np.asarray(inputs["convT_w"], np.float32)
    convT_b = np.asarray(inputs["convT_b"], np.float32)
    gauss = np.asarray(inputs["gauss"], np.float32)
    W0 = np.asarray(inputs["W0"], np.float32)
    b0 = np.asarray(inputs["b0"], np.float32)
    Wh = np.asarray(inputs["Wh"], np.float32)
    bh = np.asarray(inputs["bh"], np.float32)
    ln_g = np.asarray(inputs["ln_g"], np.float32)
    ln_b = np.asarray(inputs["ln_b"], np.float32)
    W_out = np.asarray(inputs["W_out"], np.float32)
    b_out = np.asarray(inputs["b_out"], np.float32)
    xyz = inp[:, -3:]
    lat = inp[:, :-3]
    f = (xyz + 1.0) * 0.5
    frac = f - np.clip(f.astype(np.int32), 0, 0)
    A = convT_w.transpose(2, 3, 4, 0, 1)
    lat_i = np.zeros((inp.shape[0], 512), np.float32)
    wx = [1 - frac[:, 0], frac[:, 0]]
    wy = [1 - frac[:, 1], frac[:, 1]]
    wz = [1 - frac[:, 2], frac[:, 2]]
    for di in (0, 1):
        for dj in (0, 1):
            for dk in (0, 1):
                w = (wx[di] * wy[dj] * wz[dk]).astype(np.float32)
                lat_i += (lat @ A[di, dj, dk]) * w[:, None]
    lat_i += convT_b[None, :]
    ang = 2 * np.pi * (xyz @ gauss.T)
    x = np.concatenate([lat_i, np.sin(ang), np.cos(ang)], axis=1)

    def ln(t, g, b):
        mu = t.mean(-1, keepdims=True)
        var = ((t - mu) ** 2).mean(-1, keepdims=True)
        return (t - mu) / np.sqrt(var + EPS) * g + b

    x = np.maximum(ln(x @ W0 + b0, ln_g[0], ln_b[0]), 0)
    for l in range(7):
        x = np.maximum(ln(x @ Wh[l] + bh[l], ln_g[l + 1], ln_b[l + 1]), 0)
    y = x @ W_out + b_out
    return np.concatenate([np.tanh(y[:, :1]), y[:, 1:] * 255.0], axis=1).astype(np.float32)


_NC_CACHE = {}


def _build_bass(s_core=S_CORE, repeat=1):
    """Build the per-core Bass module (SPMD; same program on all 8 cores)."""
    import concourse.bass as bass
    import concourse.bacc as bacc
    import concourse.tile as tile
    from concourse import mybir

    FP32 = mybir.dt.float32
    FP32R = mybir.dt.float32r
    AF = mybir.ActivationFunctionType
    ALU = mybir.AluOpType
    TWO_PI = float(2.0 * np.pi)
    MAGIC = 12582912.0            # 1.5 * 2^23: fp32 add/sub rounds to integer
    n_blocks = s_core // BLK

    nc = bacc.Bacc("TRN2", target_bir_lowering=False, debug=False)

    inp_d = nc.dram_tensor("inp", [s_core, 259], FP32, kind="ExternalInput").ap()
    w0p_d = nc.dram_tensor("w0p", [128, 18 * 512], FP32R, kind="ExternalInput").ap()
    whp_d = nc.dram_tensor("whp", [128, 28 * 512], FP32R, kind="ExternalInput").ap()
    swp_d = nc.dram_tensor("swp", [128, 2 * 512], FP32R, kind="ExternalInput").ap()
    ident_d = nc.dram_tensor("ident", [128, 128], FP32, kind="ExternalInput").ap()
    gaussT_d = nc.dram_tensor("gaussT", [3, 128], FP32R, kind="ExternalInput").ap()
    sel8_d = nc.dram_tensor("sel8", [8, 8 * 128], FP32R, kind="ExternalInput").ap()
    woutp_d = nc.dram_tensor("woutp", [128, 16], FP32R, kind="ExternalInput").ap()
    outT_d = nc.dram_tensor("outT", [4, s_core], FP32, kind="ExternalOutput").ap()

    def r(ap):
        return ap

    with tile.TileContext(nc) as tc:
        with (
            tc.tile_pool(name="const", bufs=1) as constp,
            tc.tile_pool(name="weights", bufs=1) as weightp,
            tc.tile_pool(name="inblk", bufs=2) as inp_pool,
            tc.tile_pool(name="acts", bufs=2) as actp,
            tc.tile_pool(name="scratch", bufs=2) as scr,
            tc.tile_pool(name="ps_t", bufs=1, space="PSUM") as ps_t,
            tc.tile_pool(name="ps_misc", bufs=2, space="PSUM") as ps_misc,
            tc.tile_pool(name="ps_gi", bufs=2, space="PSUM") as ps_gi,
        ):
            # ---- constants / weights (loaded once, resident; split into
            # chunked DMAs so they spread across queues and overlap) ----
            w0_sb = weightp.tile([128, 18, 512], FP32R)
            w0r = w0p_d.rearrange("p (c f) -> p c f", c=18)
            for ch in range(3):
                nc.sync.dma_start(
                    out=w0_sb[:, ch * 6:(ch + 1) * 6, :], in_=w0r[:, ch * 6:(ch + 1) * 6, :])
            wh_sb = weightp.tile([128, 28, 512], FP32R)
            whr = whp_d.rearrange("p (c f) -> p c f", c=28)
            for ch in range(4):
                nc.sync.dma_start(
                    out=wh_sb[:, ch * 7:(ch + 1) * 7, :], in_=whr[:, ch * 7:(ch + 1) * 7, :])
            sw_sb = weightp.tile([128, 2, 512], FP32R)
            nc.sync.dma_start(out=sw_sb, in_=swp_d.rearrange("p (c f) -> p c f", c=2))
            ident_dma = constp.tile([128, 128], FP32, name="ident_dma")
            nc.sync.dma_start(out=ident_dma, in_=ident_d)
            # DVE-gate the identity so PE transposes only ever wait on DVE
            ident_sb = constp.tile([128, 128], FP32, name="ident_sb")
            nc.vector.tensor_copy(ident_sb, ident_dma)
            gaussT_sb = constp.tile([3, 128], FP32R)
            nc.sync.dma_start(out=gaussT_sb, in_=gaussT_d)
            sel8_sb = constp.tile([8, 8, 128], FP32R)
            nc.sync.dma_start(out=sel8_sb, in_=sel8_d.rearrange("p (m f) -> p m f", m=8))
            wout_sb = weightp.tile([128, 4, 4], FP32R)
            nc.sync.dma_start(out=wout_sb, in_=woutp_d.rearrange("p (c f) -> p c f", c=4))

            inp_r = inp_d.rearrange("(b sc p) f -> b p sc f", sc=4, p=128)

            def load_block(b):
                """DMA a block in and DVE-gate it: downstream consumers then
                depend only on the DVE semaphore (PE LDW takes 1 wait max)."""
                inb0 = inp_pool.tile([128, 4, 259], FP32, tag="inb0", name="inb0")
                nc.sync.dma_start(out=inb0, in_=inp_r[b])
                inb = inp_pool.tile([128, 4, 259], FP32, tag="inb", name="inb")
                nc.vector.tensor_copy(inb, inb0)
                return inb

            for _rep in range(repeat):
              inb_next = load_block(0)
              for b in range(n_blocks):
                inb = inb_next

                # ---- transpose lat and xyz to [feature, sample] ----
                latT = scr.tile([128, 2, BLK], FP32, tag="latT", bufs=1)
                xyzT = scr.tile([3, BLK], FP32R, tag="xyzT")
                for sc in range(4):
                    for fc in range(2):
                        tp = ps_misc.tile([128, 128], FP32, tag="mt")
                        nc.tensor.transpose(
                            tp, inb[:, sc, fc * 128:(fc + 1) * 128], ident_sb
                        )
                        nc.vector.tensor_copy(latT[:, fc, sc * 128:(sc + 1) * 128], tp)
                    tp3 = ps_misc.tile([3, 128], FP32, tag="mt")
                    nc.tensor.transpose(tp3, inb[:, sc, 256:259], ident_sb)
                    nc.vector.tensor_copy(xyzT[:, sc * 128:(sc + 1) * 128], tp3)

                # ---- corner weights in sample layout (whole block), then transpose ----
                w8T = scr.tile([8, BLK], FP32R, tag="w8T", bufs=1)
                f3 = scr.tile([128, 4, 3], FP32, tag="f3")
                nc.vector.tensor_scalar(
                    out=f3, in0=inb[:, :, 256:259],
                    scalar1=0.5, scalar2=0.5, op0=ALU.mult, op1=ALU.add,
                )
                om3 = scr.tile([128, 4, 3], FP32, tag="om3")
                nc.vector.tensor_scalar(
                    out=om3, in0=f3, scalar1=1.0, scalar2=-1.0,
                    op0=ALU.subtract, op1=ALU.mult,
                )
                wxy = scr.tile([128, 4, 4], FP32, tag="wxy")
                w8s = scr.tile([128, 4, 8], FP32, tag="w8s")
                for di in (0, 1):
                    xs = (f3 if di else om3)[:, :, 0:1]
                    for dj in (0, 1):
                        ys = (f3 if dj else om3)[:, :, 1:2]
                        nc.vector.tensor_tensor(
                            out=wxy[:, :, di * 2 + dj:di * 2 + dj + 1],
                            in0=xs, in1=ys, op=ALU.mult,
                        )
                for m in range(8):
                    di, dj, dk = (m >> 2) & 1, (m >> 1) & 1, m & 1
                    zsl = (f3 if dk else om3)[:, :, 2:3]
                    nc.vector.tensor_tensor(
                        out=w8s[:, :, m:m + 1],
                        in0=wxy[:, :, di * 2 + dj:di * 2 + dj + 1],
                        in1=zsl, op=ALU.mult,
                    )
                for sc in range(4):
                    tp8 = ps_misc.tile([8, 128], FP32, tag="mt")
                    nc.tensor.transpose(tp8, w8s[:, sc, :], ident_sb)
                    nc.vector.tensor_copy(w8T[:, sc * 128:(sc + 1) * 128], tp8)

                if b + 1 < n_blocks:
                    inb_next = load_block(b + 1)

                # ---- fourier angle, range-reduced to [-0.5, 0.5] ----
                angp = ps_misc.tile([128, BLK], FP32, tag="mt")
                nc.tensor.matmul(angp, r(gaussT_sb), r(xyzT), start=True, stop=True)
                ang_sb = scr.tile([128, BLK], FP32, tag="rr", bufs=3, name="ang_sb")
                nc.vector.tensor_copy(ang_sb, angp)
                # zs = ang - round(ang); zc = a25 - round(a25), a25 = ang + 0.25
                zs_r = scr.tile([128, BLK], FP32, tag="rr", bufs=3, name="zs_r")
                nc.vector.tensor_scalar(
                    out=zs_r, in0=ang_sb, scalar1=MAGIC, scalar2=MAGIC,
                    op0=ALU.add, op1=ALU.subtract,
                )
                zs = scr.tile([128, BLK], FP32, tag="zs", bufs=1)
                nc.vector.tensor_sub(zs, ang_sb, zs_r)
                a25 = scr.tile([128, BLK], FP32, tag="a25", bufs=1)
                nc.vector.tensor_scalar_add(out=a25, in0=ang_sb, scalar1=0.25)
                zc_r = scr.tile([128, BLK], FP32, tag="rr", bufs=3, name="zc_r")
                nc.vector.tensor_scalar(
                    out=zc_r, in0=a25, scalar1=MAGIC, scalar2=MAGIC,
                    op0=ALU.add, op1=ALU.subtract,
                )
                zc = scr.tile([128, BLK], FP32, tag="zc", bufs=1)
                nc.vector.tensor_sub(zc, a25, zc_r)

                # ---- layer 0: build u chunks incrementally + matmul ----
                psums = [ps_t.tile([128, BLK], FP32, tag=f"pt{mc}", name=f"pt{mc}") for mc in range(4)]
                uch_i = 0

                def l0_accum(u_ap, last=False):
                    nonlocal uch_i
                    for mc in range(4):
                        nc.tensor.matmul(
                            psums[mc],
                            r(w0_sb[:, uch_i, mc * 128:(mc + 1) * 128]),
                            r(u_ap),
                            start=(uch_i == 0), stop=last,
                        )
                    uch_i += 1

                for m in range(8):
                    bc = ps_misc.tile([128, BLK], FP32, tag="mt")
                    nc.tensor.matmul(
                        bc, r(sel8_sb[:, m, :]), r(w8T), start=True, stop=True
                    )
                    for kc in range(2):
                        uch = scr.tile([128, BLK], FP32R, tag="uch")
                        nc.vector.tensor_tensor(
                            out=uch, in0=latT[:, kc, :], in1=bc, op=ALU.mult
                        )
                        l0_accum(uch)
                ffs = scr.tile([128, BLK], FP32R, tag="uch")
                nc.scalar.activation(out=ffs, in_=zs, func=AF.Sin, scale=TWO_PI)
                l0_accum(ffs)
                ffc = scr.tile([128, BLK], FP32R, tag="uch")
                nc.scalar.activation(out=ffc, in_=zc, func=AF.Sin, scale=TWO_PI)
                l0_accum(ffc, last=True)

                # ---- LN layers: relu; stats only for the last two.
                # Stats matmuls are emitted AFTER the next layer's main
                # matmuls so the PE never waits on the ACT squares.
                x_cur = None
                pending_stats = None
                gp6 = gp7 = None

                def emit_stats():
                    nonlocal gp6, gp7, pending_stats
                    if pending_stats is None:
                        return
                    jj, sq_t = pending_stats
                    gp = ps_gi.tile([128, BLK], FP32, tag="gp", name=f"gp{jj}")
                    for mc in range(4):
                        nc.tensor.matmul(
                            gp, r(sw_sb[:, jj - 6, mc * 128:(mc + 1) * 128]),
                            r(sq_t[:, mc, :]), start=(mc == 0), stop=(mc == 3),
                        )
                    if jj == 6:
                        gp6 = gp
                    else:
                        gp7 = gp
                    pending_stats = None

                for j in range(N_LAYERS):
                    if j > 0:
                        psums = [
                            ps_t.tile([128, BLK], FP32, tag=f"pt{mc}", name=f"pt{mc}")
                            for mc in range(4)
                        ]
                        for mc in range(4):
                            for kc in range(4):
                                nc.tensor.matmul(
                                    psums[mc],
                                    r(wh_sb[:, (j - 1) * 4 + kc,
                                            mc * 128:(mc + 1) * 128]),
                                    r(x_cur[:, kc, :]),
                                    start=(kc == 0), stop=(kc == 3),
                                )
                    emit_stats()
                    x_next = actp.tile([128, 4, BLK], FP32R, tag="xn")
                    for mc in range(4):
                        nc.scalar.activation(
                            out=x_next[:, mc, :], in_=psums[mc], func=AF.Relu
                        )
                    if j >= 6:
                        sq = scr.tile([128, 4, BLK], FP32R, tag="sq", bufs=2)
                        for mc in range(4):
                            nc.scalar.activation(
                                out=sq[:, mc, :], in_=psums[mc], func=AF.Square
                            )
                        pending_stats = (j, sq)
                    x_cur = x_next

                # ---- output layer ----
                yp = ps_gi.tile([4, BLK], FP32, tag="gp", name="yp")
                for kc in range(4):
                    nc.tensor.matmul(
                        yp, r(wout_sb[:, kc, :]), r(x_cur[:, kc, :]),
                        start=(kc == 0), stop=(kc == 3),
                    )
                emit_stats()

                # gi2 = gp7 + eps*gp6, then finalize this block in place:
                # out = [tanh(yhat/gi), 255*yhat/gi] with gi = sqrt(gi2)
                g6 = scr.tile([4, BLK], FP32, tag="g6")
                nc.vector.tensor_copy(g6, gp6[0:4, :])
                g4 = scr.tile([4, BLK], FP32, tag="g4")
                nc.vector.scalar_tensor_tensor(
                    out=g4, in0=g6, scalar=EPS, in1=gp7[0:4, :],
                    op0=ALU.mult, op1=ALU.add,
                )
                sg = scr.tile([4, BLK], FP32, tag="sg")
                nc.scalar.activation(out=sg, in_=g4, func=AF.Sqrt)
                rg = scr.tile([4, BLK], FP32, tag="rg")
                nc.vector.reciprocal(out=rg, in_=sg)
                yv = scr.tile([4, BLK], FP32, tag="yv")
                nc.vector.tensor_tensor(out=yv, in0=yp, in1=rg, op=ALU.mult)
                nc.scalar.mul(out=yv, in_=yv, mul=255.0)
                nc.scalar.activation(
                    out=yv[0:1, :], in_=yv[0:1, :], func=AF.Tanh, scale=1.0 / 255.0
                )
                nc.sync.dma_start(out=outT_d[:, b * BLK:(b + 1) * BLK], in_=yv)

    nc.compile()
    return nc


def kernel(**inputs):
    if _general_case_needed(inputs):
        return _numpy_fallback(inputs)

    from concourse.bass_utils import run_bass_kernel_spmd

    pre = _precompute(inputs)
    inp = np.ascontiguousarray(np.asarray(inputs["input"], np.float32))

    if "nc" not in _NC_CACHE:
        _NC_CACHE["nc"] = _build_bass()
    nc = _NC_CACHE["nc"]

    in_maps = [
        {
            "inp": np.ascontiguousarray(inp[c * S_CORE:(c + 1) * S_CORE]),
            "w0p": pre["w0p"], "whp": pre["whp"], "swp": pre["swp"],
            "ident": pre["ident"], "gaussT": pre["gaussT"],
            "sel8": pre["sel8"], "woutp": pre["woutp"],
        }
        for c in range(N_CORES)
    ]

    res = run_bass_kernel_spmd(
        nc, in_maps, core_ids=list(range(N_CORES)),
        trace=bool(int(os.environ.get("KERNEL_TRACE", "0"))),
    )
    kernel.last_results = res
    outs = [res.results[c]["outT"] for c in range(N_CORES)]
    return np.ascontiguousarray(
        np.concatenate([o.T for o in outs], axis=0).astype(np.float32)
    )



# revision 13
# speedup vs baseline: 1.2028x; 1.2028x over previous
"""Trainium2 Bass kernel for nn_Decoder (latent-grid decoder MLP).

Contract: kernel(**inputs) takes the FULL unsharded inputs (as produced by
setup_inputs()) and returns the FULL [65536, 4] float32 output. Internally the
65536 points are sharded across 8 NeuronCores (pure data parallel); the small
weights are replicated.

Algorithm (mathematically equivalent to the reference):
  - G=2 trilinear interp of a per-sample 2x2x2 grid always lands in cell
    (0,0,0) (indices clip to [0, G-2] = [0,0]), so
    lat_i = sum_m w_m(xyz) * (lat @ A_m), A_m = convT_w[:, :, di, dj, dk].
  - The interp + Fourier features + first MLP layer fold into one matmul:
    u = [w_0*lat, ..., w_7*lat, sin(2 pi ang), cos(2 pi ang)]  (2304 dims),
    h0 = u @ M0 with M0 = [A_stack @ W0_top; W0_sin; W0_cos] (host-folded).
  - LayerNorm mean-subtraction folds into the weights (column centering);
    ln gamma folds in too. The per-sample rstd is deferred via LN's positive
    scale invariance: activations stay unnormalized, and the final scale is
    1/sqrt(gi2) with gi2 = ssq(h7)/512 (the eps*gi2_6 correction term is
    ~1e-4 relative and is dropped; a tiny bias guards ssq == 0).
    Requires all biases and ln_b == 0 (true for this model; a numpy fallback
    covers the general case).
  - ssq(h7) is accumulated by the TensorEngine itself (weighted-ones
    stationary operand, M=4) into a [4, S] PSUM tile; the host folds a
    constant c into those weights so the Dsqrt activation-table input is
    centered near 0.25, and 255/sqrt(.) comes out of one ACT Dsqrt op
    (Dsqrt(x) = 1/(2 sqrt(x))) plus one DVE multiply.

Schedule (the reason this version is fast): the per-block preamble
(transposes, corner-weight chain, Fourier angle) for block b+1 is emitted in
the middle of block b's hidden-layer phase, so the PE never drains between
blocks and the HAM clock governor stays at full rate. Input DMAs are issued
two blocks ahead and before the (much larger) weight DMAs so compute starts
~6us into the kernel instead of ~42us. Activations live in [feature, sample]
layout; matmuls run as fp32r (full PE rate at N=512).
"""

import os
import numpy as np

N_CORES = 8
N_TOTAL = 65536
S_CORE = N_TOTAL // N_CORES          # 8192 samples per core
BLK = 512                            # samples per block
N_BLOCKS = S_CORE // BLK             # 16
EPS = 1e-5
N_LAYERS = 8                         # LN+relu layers (layer0 + 7 hidden)


def _precompute(inputs):
    """Host-side weight folding. Returns dict of constant arrays (fp32)."""
    convT_w = np.asarray(inputs["convT_w"], np.float32)
    W0 = np.asarray(inputs["W0"], np.float32)
    Wh = np.asarray(inputs["Wh"], np.float32)
    ln_g = np.asarray(inputs["ln_g"], np.float32)
    gauss = np.asarray(inputs["gauss"], np.float32)
    W_out = np.asarray(inputs["W_out"], np.float32)

    # A_stack[m*256+i, c] = convT_w[i, c, di, dj, dk], m = 4*di + 2*dj + dk
    A_stack = convT_w.transpose(2, 3, 4, 0, 1).reshape(8 * 256, 512)
    M0 = np.concatenate([A_stack @ W0[:512], W0[512:640], W0[640:768]], axis=0)

    def center_scale(W, g):
        Wc = W - W.mean(axis=1, keepdims=True)
        return np.ascontiguousarray(Wc * g[None, :], np.float32)

    W_eff = [center_scale(M0, ln_g[0])] + [
        center_scale(Wh[l], ln_g[l + 1]) for l in range(7)
    ]

    # pack each layer's weights as [128, n_kchunks, 512]
    def pack(W):
        K = W.shape[0]
        kc = K // 128
        return W.reshape(kc, 128, 512).transpose(1, 0, 2).reshape(128, kc * 512)

    w0p = np.ascontiguousarray(pack(W_eff[0]))                       # [128, 18*512]
    whp = np.ascontiguousarray(
        np.concatenate([pack(W) for W in W_eff[1:]], axis=1))        # [128, 28*512]
    # stats lhsT (layer 7 only, M=4): col (mc*4 + m) = 1/(512*g7[mc*128+k]^2)
    swv = (1.0 / (512.0 * ln_g[7] ** 2)).astype(np.float32)
    sw4 = np.empty((128, 16), np.float32)
    for mc in range(4):
        for m in range(4):
            sw4[:, mc * 4 + m] = swv[mc * 128:(mc + 1) * 128]

    return {
        "w0p": w0p,
        "whp": whp,
        "swp": np.ascontiguousarray(sw4),
        "ident": np.eye(128, dtype=np.float32),
        "gaussT": np.ascontiguousarray(gauss.T.astype(np.float32)),  # [3, 128]
        "sel8": np.ascontiguousarray(
            np.kron(np.eye(8, dtype=np.float32), np.ones((1, 128), np.float32))),
        "woutp": np.ascontiguousarray(
            W_out.reshape(4, 128, 4).transpose(1, 0, 2).reshape(128, 16)),
    }


def _general_case_needed(inputs):
    z = lambda a: bool(np.all(np.asarray(a) == 0))
    return not (
        z(inputs["convT_b"]) and z(inputs["b0"]) and z(inputs["bh"])
        and z(inputs["ln_b"]) and z(inputs["b_out"])
        and bool(np.all(np.abs(np.asarray(inputs["ln_g"])) > 1e-3))
    )


def _numpy_fallback(inputs):
    """Reference in numpy (slow; only for inputs outside the fast path)."""
    inp = np.asarray(inputs["input"], np.float32)
    convT_w = np.asarray(inputs["convT_w"], np.float32)
    convT_b = np.asarray(inputs["convT_b"], np.float32)
    gauss = np.asarray(inputs["gauss"], np.float32)
    W0 = np.asarray(inputs["W0"], np.float32)
    b0 = np.asarray(inputs["b0"], np.float32)
    Wh = np.asarray(inputs["Wh"], np.float32)
    bh = np.asarray(inputs["bh"], np.float32)
    ln_g = np.asarray(inputs["ln_g"], np.float32)
    ln_b = np.asarray(inputs["ln_b"], np.float32)
    W_out = np.asarray(inputs["W_out"], np.float32)
    b_out = np.asarray(inputs["b_out"], np.float32)
    xyz = inp[:, -3:]
    lat = inp[:, :-3]
    f = (xyz + 1.0) * 0.5
    frac = f - np.clip(f.astype(np.int32), 0, 0)
    A = convT_w.transpose(2, 3, 4, 0, 1)
    lat_i = np.zeros((inp.shape[0], 512), np.float32)
    wx = [1 - frac[:, 0], frac[:, 0]]
    wy = [1 - frac[:, 1], frac[:, 1]]
    wz = [1 - frac[:, 2], frac[:, 2]]
    for di in (0, 1):
        for dj in (0, 1):
            for dk in (0, 1):
                w = (wx[di] * wy[dj] * wz[dk]).astype(np.float32)
                lat_i += (lat @ A[di, dj, dk]) * w[:, None]
    lat_i += convT_b[None, :]
    ang = 2 * np.pi * (xyz @ gauss.T)
    x = np.concatenate([lat_i, np.sin(ang), np.cos(ang)], axis=1)

    def ln(t, g, b):
        mu = t.mean(-1, keepdims=True)
        var = ((t - mu) ** 2).mean(-1, keepdims=True)
        return (t - mu) / np.sqrt(var + EPS) * g + b

    x = np.maximum(ln(x @ W0 + b0, ln_g[0], ln_b[0]), 0)
    for l in range(7):
        x = np.maximum(ln(x @ Wh[l] + bh[l], ln_g[l + 1], ln_b[l + 1]), 0)
    y = x @ W_out + b_out
    return np.concatenate([np.tanh(y[:, :1]), y[:, 1:] * 255.0], axis=1).astype(np.float32)


_NC_CACHE = {}


def _build_bass(s_core=S_CORE):
    """Build the per-core Bass module (SPMD; same program on all 8 cores)."""
    import concourse.bass as bass
    import concourse.bacc as bacc
    import concourse.tile as tile
    from concourse import mybir

    FP32 = mybir.dt.float32
    FP32R = mybir.dt.float32r
    I32 = mybir.dt.int32
    AF = mybir.ActivationFunctionType
    ALU = mybir.AluOpType
    TWO_PI = float(2.0 * np.pi)
    MAGIC = 12582912.0            # 1.5 * 2^23: fp32 add/sub rounds to integer
    RSQRT_SEED = 0x5F3759DF       # fp32 fast-inverse-sqrt seed constant
    n_blocks = s_core // BLK

    nc = bacc.Bacc("TRN2", target_bir_lowering=False, debug=False)

    inp_d = nc.dram_tensor("inp", [s_core, 259], FP32, kind="ExternalInput").ap()
    w0p_d = nc.dram_tensor("w0p", [128, 18 * 512], FP32R, kind="ExternalInput").ap()
    whp_d = nc.dram_tensor("whp", [128, 28 * 512], FP32R, kind="ExternalInput").ap()
    swp_d = nc.dram_tensor("swp", [128, 16], FP32R, kind="ExternalInput").ap()
    ident_d = nc.dram_tensor("ident", [128, 128], FP32R, kind="ExternalInput").ap()
    gaussT_d = nc.dram_tensor("gaussT", [3, 128], FP32R, kind="ExternalInput").ap()
    sel8_d = nc.dram_tensor("sel8", [8, 8 * 128], FP32R, kind="ExternalInput").ap()
    woutp_d = nc.dram_tensor("woutp", [128, 16], FP32R, kind="ExternalInput").ap()
    outT_d = nc.dram_tensor("outT", [4, s_core], FP32, kind="ExternalOutput").ap()

    def r(ap):
        return ap

    with tile.TileContext(nc) as tc:
        with (
            tc.tile_pool(name="const", bufs=1) as constp,
            tc.tile_pool(name="weights", bufs=1) as weightp,
            tc.tile_pool(name="inblk", bufs=2) as inp_pool,
            tc.tile_pool(name="acts", bufs=2) as actp,
            tc.tile_pool(name="scratch", bufs=2) as scr,
            tc.tile_pool(name="ps_t", bufs=1, space="PSUM") as ps_t,
            tc.tile_pool(name="ps_misc", bufs=2, space="PSUM") as ps_misc,
            tc.tile_pool(name="ps_gi", bufs=2, space="PSUM") as ps_gi,
        ):
            # ---- small constants first (so they beat the weight DMAs) ----
            ident_dma = constp.tile([128, 128], FP32R, name="ident_dma")
            nc.sync.dma_start(out=ident_dma, in_=ident_d)
            # DVE-gate the identity so PE transposes only ever wait on DVE
            ident_sb = constp.tile([128, 128], FP32R, name="ident_sb")
            nc.vector.tensor_copy(ident_sb, ident_dma)
            gaussT_sb = constp.tile([3, 128], FP32R)
            nc.sync.dma_start(out=gaussT_sb, in_=gaussT_d)
            sel8_sb = constp.tile([8, 8, 128], FP32R)
            nc.sync.dma_start(out=sel8_sb, in_=sel8_d.rearrange("p (m f) -> p m f", m=8))
            wout_sb = weightp.tile([128, 4, 4], FP32R)
            nc.sync.dma_start(out=wout_sb, in_=woutp_d.rearrange("p (c f) -> p c f", c=4))
            sw_sb = weightp.tile([128, 16], FP32R)
            nc.sync.dma_start(out=sw_sb, in_=swp_d)

            inp_r = inp_d.rearrange("(b sc p) f -> b p sc f", sc=4, p=128)

            def load_start(b):
                """Issue the input DMA for block b (returns the landing tile)."""
                inb0 = inp_pool.tile([128, 4, 259], FP32, tag="inb0", name="inb0")
                nc.sync.dma_start(out=inb0, in_=inp_r[b])
                return inb0

            # prefetch the first two input blocks before the weight DMAs
            inb0_tiles = {0: load_start(0), 1: load_start(1)}

            # ---- weights, in consumption order ----
            w0_sb = weightp.tile([128, 18, 512], FP32R)
            w0r = w0p_d.rearrange("p (c f) -> p c f", c=18)
            for ch in range(3):
                nc.sync.dma_start(
                    out=w0_sb[:, ch * 6:(ch + 1) * 6, :], in_=w0r[:, ch * 6:(ch + 1) * 6, :])
            wh_sb = weightp.tile([128, 28, 512], FP32R)
            whr = whp_d.rearrange("p (c f) -> p c f", c=28)
            for ch in range(4):
                nc.sync.dma_start(
                    out=wh_sb[:, ch * 7:(ch + 1) * 7, :], in_=whr[:, ch * 7:(ch + 1) * 7, :])

            def preamble(b):
                """Emit block b's input-side prep. Called during block b-1's
                hidden phase (or standalone for b == 0): DVE corner-weight
                chain, PE transposes (fp32r, 1.5 c/r), Fourier angle + range
                reduction, and the sin/cos feature tiles."""
                inb = inp_pool.tile([128, 4, 259], FP32R, tag="inb", name="inb")
                nc.vector.tensor_copy(inb, inb0_tiles.pop(b))

                # corner weights in sample layout (DVE only; needs just inb)
                f3 = scr.tile([128, 4, 3], FP32R, tag="f3")
                nc.vector.tensor_scalar(
                    out=f3, in0=inb[:, :, 256:259],
                    scalar1=0.5, scalar2=0.5, op0=ALU.mult, op1=ALU.add,
                )
                om3 = scr.tile([128, 4, 3], FP32R, tag="om3")
                nc.vector.tensor_scalar(
                    out=om3, in0=f3, scalar1=1.0, scalar2=-1.0,
                    op0=ALU.subtract, op1=ALU.mult,
                )
                wxy = scr.tile([128, 4, 4], FP32R, tag="wxy")
                w8s = scr.tile([128, 4, 8], FP32R, tag="w8s")
                for di in (0, 1):
                    xs = (f3 if di else om3)[:, :, 0:1]
                    for dj in (0, 1):
                        ys = (f3 if dj else om3)[:, :, 1:2]
                        nc.vector.tensor_tensor(
                            out=wxy[:, :, di * 2 + dj:di * 2 + dj + 1],
                            in0=xs, in1=ys, op=ALU.mult,
                        )
                for m in range(8):
                    di, dj, dk = (m >> 2) & 1, (m >> 1) & 1, m & 1
                    zsl = (f3 if dk else om3)[:, :, 2:3]
                    nc.vector.tensor_tensor(
                        out=w8s[:, :, m:m + 1],
                        in0=wxy[:, :, di * 2 + dj:di * 2 + dj + 1],
                        in1=zsl, op=ALU.mult,
                    )

                # transposes to [feature, sample] (fp32r: 1.5 cycles/row)
                latT = scr.tile([128, 2, BLK], FP32R, tag="latT", bufs=2)
                xyzT = scr.tile([3, BLK], FP32R, tag="xyzT", bufs=2)
                for sc in range(4):
                    for fc in range(2):
                        tp = ps_misc.tile([128, 128], FP32R, tag="mt")
                        nc.tensor.transpose(
                            tp, inb[:, sc, fc * 128:(fc + 1) * 128], ident_sb
                        )
                        nc.vector.tensor_copy(latT[:, fc, sc * 128:(sc + 1) * 128], tp)
                    tp3 = ps_misc.tile([3, 128], FP32R, tag="mt")
                    nc.tensor.transpose(tp3, inb[:, sc, 256:259], ident_sb)
                    nc.vector.tensor_copy(xyzT[:, sc * 128:(sc + 1) * 128], tp3)
                w8T = scr.tile([8, BLK], FP32R, tag="w8T", bufs=2)
                for sc in range(4):
                    tp8 = ps_misc.tile([8, 128], FP32R, tag="mt")
                    nc.tensor.transpose(tp8, w8s[:, sc, :], ident_sb)
                    nc.vector.tensor_copy(w8T[:, sc * 128:(sc + 1) * 128], tp8)

                # fourier angle, range-reduced to [-0.5, 0.5]
                angp = ps_misc.tile([128, BLK], FP32, tag="mt")
                nc.tensor.matmul(angp, r(gaussT_sb), r(xyzT), start=True, stop=True)
                ang_sb = scr.tile([128, BLK], FP32, tag="rr", bufs=3, name="ang_sb")
                nc.vector.tensor_copy(ang_sb, angp)
                zs_r = scr.tile([128, BLK], FP32, tag="rr", bufs=3, name="zs_r")
                nc.vector.tensor_scalar(
                    out=zs_r, in0=ang_sb, scalar1=MAGIC, scalar2=MAGIC,
                    op0=ALU.add, op1=ALU.subtract,
                )
                zs = scr.tile([128, BLK], FP32, tag="zs", bufs=1)
                nc.vector.tensor_sub(zs, ang_sb, zs_r)
                a25 = scr.tile([128, BLK], FP32, tag="a25", bufs=1)
                nc.vector.tensor_scalar_add(out=a25, in0=ang_sb, scalar1=0.25)
                zc_r = scr.tile([128, BLK], FP32, tag="rr", bufs=3, name="zc_r")
                nc.vector.tensor_scalar(
                    out=zc_r, in0=a25, scalar1=MAGIC, scalar2=MAGIC,
                    op0=ALU.add, op1=ALU.subtract,
                )
                zc = scr.tile([128, BLK], FP32, tag="zc", bufs=1)
                nc.vector.tensor_sub(zc, a25, zc_r)
                ffs = scr.tile([128, BLK], FP32R, tag="ff", bufs=2, name="ffs")
                nc.scalar.activation(out=ffs, in_=zs, func=AF.Sin, scale=TWO_PI)
                ffc = scr.tile([128, BLK], FP32R, tag="ff", bufs=2, name="ffc")
                nc.scalar.activation(out=ffc, in_=zc, func=AF.Sin, scale=TWO_PI)
                return latT, w8T, ffs, ffc

            def make_finalize(yp, gp, b):
                """Deferred block finalize: rg = 1/sqrt(gp) via the fp32
                bit-trick seed + 2 Newton steps (all DVE; keeps the ACT
                engine on a single activation table for the whole kernel),
                then yv = 255 * yp * rg, tanh on row 0, and the output DMA.
                Emitted a few uch-products into the NEXT block's layer 0 so
                it never head-of-line-blocks that block's DVE feed."""
                def fin():
                    sd = scr.tile([4, BLK], FP32, tag="nr_sd", bufs=1)
                    t1 = scr.tile([4, BLK], FP32, tag="nr_t1", bufs=1)
                    t2 = scr.tile([4, BLK], FP32, tag="nr_t2", bufs=1)
                    y1 = scr.tile([4, BLK], FP32, tag="nr_y1", bufs=1)
                    yv = scr.tile([4, BLK], FP32, tag="yv", bufs=1)
                    # seed bits = RSQRT_SEED - (bits(gp) >> 1)
                    nc.vector.tensor_scalar(
                        out=t1[0:4, :].bitcast(I32), in0=gp[0:4, :].bitcast(I32),
                        scalar1=1, scalar2=None, op0=ALU.logical_shift_right,
                    )
                    nc.vector.tensor_scalar(
                        out=sd[0:4, :].bitcast(I32), in0=t1[0:4, :].bitcast(I32),
                        scalar1=RSQRT_SEED, scalar2=-1,
                        op0=ALU.subtract, op1=ALU.mult,
                    )
                    # Newton 1: y1 = sd * (1.5 - 0.5 * gp * sd^2)
                    nc.vector.tensor_tensor(out=t1, in0=sd, in1=sd, op=ALU.mult)
                    nc.vector.scalar_tensor_tensor(
                        out=t2, in0=t1, scalar=-0.5, in1=gp[0:4, :],
                        op0=ALU.mult, op1=ALU.mult,
                    )
                    nc.vector.tensor_scalar_add(out=t2, in0=t2, scalar1=1.5)
                    nc.vector.tensor_tensor(out=y1, in0=sd, in1=t2, op=ALU.mult)
                    # Newton 2: rg = y1 * (1.5 - 0.5 * gp * y1^2)
                    nc.vector.tensor_tensor(out=t1, in0=y1, in1=y1, op=ALU.mult)
                    nc.vector.scalar_tensor_tensor(
                        out=t2, in0=t1, scalar=-0.5, in1=gp[0:4, :],
                        op0=ALU.mult, op1=ALU.mult,
                    )
                    nc.vector.tensor_scalar_add(out=t2, in0=t2, scalar1=1.5)
                    nc.vector.tensor_tensor(out=t1, in0=y1, in1=t2, op=ALU.mult)
                    # yv = (yp * 255) * rg ; out row0 = tanh(yv0 / 255)
                    nc.vector.scalar_tensor_tensor(
                        out=yv, in0=yp, scalar=255.0, in1=t1,
                        op0=ALU.mult, op1=ALU.mult,
                    )
                    nc.scalar.activation(
                        out=yv[0:1, :], in_=yv[0:1, :], func=AF.Tanh,
                        scale=1.0 / 255.0,
                    )
                    nc.sync.dma_start(
                        out=outT_d[:, b * BLK:(b + 1) * BLK], in_=yv)
                return fin

            pre_next = preamble(0)
            fin_prev = None

            for b in range(n_blocks):
                latT, w8T, ffs, ffc = pre_next
                if b + 2 < n_blocks:
                    inb0_tiles[b + 2] = load_start(b + 2)

                # ---- layer 0: build u chunks incrementally + matmul ----
                psums = [ps_t.tile([128, BLK], FP32, tag=f"pt{mc}", name=f"pt{mc}")
                         for mc in range(4)]
                uch_i = 0

                def l0_accum(u_ap, last=False):
                    nonlocal uch_i
                    for mc in range(4):
                        nc.tensor.matmul(
                            psums[mc],
                            r(w0_sb[:, uch_i, mc * 128:(mc + 1) * 128]),
                            r(u_ap),
                            start=(uch_i == 0), stop=last,
                        )
                    uch_i += 1

                for m in range(8):
                    bc = ps_misc.tile([128, BLK], FP32, tag="mt")
                    nc.tensor.matmul(
                        bc, r(sel8_sb[:, m, :]), r(w8T), start=True, stop=True
                    )
                    for kc in range(2):
                        uch = scr.tile([128, BLK], FP32R, tag="uch", bufs=6)
                        nc.vector.tensor_tensor(
                            out=uch, in0=latT[:, kc, :], in1=bc, op=ALU.mult
                        )
                        l0_accum(uch)
                    if m == 1 and fin_prev is not None:
                        fin_prev()
                        fin_prev = None
                l0_accum(ffs)
                l0_accum(ffc, last=True)

                # ---- hidden LN+relu layers; block b+1's preamble is emitted
                # after layer 2 so every engine stream stays deep ----
                x_cur = None
                sq = None
                for j in range(N_LAYERS):
                    if j > 0:
                        psums = [
                            ps_t.tile([128, BLK], FP32, tag=f"pt{mc}", name=f"pt{mc}")
                            for mc in range(4)
                        ]
                        for mc in range(4):
                            for kc in range(4):
                                nc.tensor.matmul(
                                    psums[mc],
                                    r(wh_sb[:, (j - 1) * 4 + kc,
                                            mc * 128:(mc + 1) * 128]),
                                    r(x_cur[:, kc, :]),
                                    start=(kc == 0), stop=(kc == 3),
                                )
                    x_next = actp.tile([128, 4, BLK], FP32R, tag="xn")
                    for mc in range(4):
                        nc.scalar.activation(
                            out=x_next[:, mc, :], in_=psums[mc], func=AF.Relu
                        )
                    if j == 7:
                        sq = scr.tile([128, 4, BLK], FP32R, tag="sq", bufs=1)
                        for mc in range(4):
                            nc.scalar.activation(
                                out=sq[:, mc, :], in_=psums[mc], func=AF.Square
                            )
                    x_cur = x_next
                    if j == 2 and b + 1 < n_blocks:
                        pre_next = preamble(b + 1)

                # ---- output layer, then deferred-LN stats (sq is ready by
                # the time the wout matmuls finish) ----
                yp = ps_gi.tile([4, BLK], FP32, tag="gp", name="yp")
                for kc in range(4):
                    nc.tensor.matmul(
                        yp, r(wout_sb[:, kc, :]), r(x_cur[:, kc, :]),
                        start=(kc == 0), stop=(kc == 3),
                    )
                gp = ps_gi.tile([4, BLK], FP32, tag="gp", name="gp7")
                for mc in range(4):
                    nc.tensor.matmul(
                        gp, r(sw_sb[:, mc * 4:(mc + 1) * 4]),
                        r(sq[:, mc, :]), start=(mc == 0), stop=(mc == 3),
                    )
                fin_prev = make_finalize(yp, gp, b)

            fin_prev()

    nc.compile()
    return nc


def kernel(**inputs):
    if _general_case_needed(inputs):
        return _numpy_fallback(inputs)

    from concourse.bass_utils import run_bass_kernel_spmd

    pre = _precompute(inputs)
    inp = np.ascontiguousarray(np.asarray(inputs["input"], np.float32))

    if "nc" not in _NC_CACHE:
        _NC_CACHE["nc"] = _build_bass()
    nc = _NC_CACHE["nc"]

    in_maps = [
        {
            "inp": np.ascontiguousarray(inp[c * S_CORE:(c + 1) * S_CORE]),
            "w0p": pre["w0p"], "whp": pre["whp"], "swp": pre["swp"],
            "ident": pre["ident"], "gaussT": pre["gaussT"],
            "sel8": pre["sel8"], "woutp": pre["woutp"],
        }
        for c in range(N_CORES)
    ]

    res = run_bass_kernel_spmd(
        nc, in_maps, core_ids=list(range(N_CORES)),
        trace=bool(int(os.environ.get("KERNEL_TRACE", "0"))),
    )
    kernel.last_results = res
    outs = [res.results[c]["outT"] for c in range(N_CORES)]
    return np.ascontiguousarray(
        np.concatenate([o.T for o in outs], axis=0).astype(np.float32)
    )


# revision 20
# speedup vs baseline: 1.2391x; 1.0301x over previous
"""Trainium2 Bass kernel for nn_Decoder (latent-grid decoder MLP).

Contract: kernel(**inputs) takes the FULL unsharded inputs (as produced by
setup_inputs()) and returns the FULL [65536, 4] float32 output. Internally the
65536 points are sharded across 8 NeuronCores (pure data parallel); the small
weights are replicated.

Algorithm (mathematically equivalent to the reference):
  - G=2 trilinear interp of a per-sample 2x2x2 grid always lands in cell
    (0,0,0) (indices clip to [0, G-2] = [0,0]), so
    lat_i = sum_m w_m(xyz) * (lat @ A_m), A_m = convT_w[:, :, di, dj, dk].
  - The interp + Fourier features + first MLP layer fold into one matmul:
    u = [w_0*lat, ..., w_7*lat, sin(2 pi ang), cos(2 pi ang)]  (2304 dims),
    h0 = u @ M0 with M0 = [A_stack @ W0_top; W0_sin; W0_cos] (host-folded).
  - LayerNorm mean-subtraction folds into the weights (column centering);
    ln gamma folds in too. The per-sample rstd is deferred via LN's positive
    scale invariance: activations stay unnormalized, and the final scale is
    1/sqrt(gi2) with gi2 = ssq(h7)/512 (the eps*gi2_6 correction term is
    ~1e-4 relative and is dropped; a tiny bias guards ssq == 0).
    Requires all biases and ln_b == 0 (true for this model; a numpy fallback
    covers the general case).
  - ssq(h7) is accumulated by the TensorEngine itself (weighted-ones
    stationary operand, M=4) into a [4, S] PSUM tile; the host folds a
    constant c into those weights so the Dsqrt activation-table input is
    centered near 0.25, and 255/sqrt(.) comes out of one ACT Dsqrt op
    (Dsqrt(x) = 1/(2 sqrt(x))) plus one DVE multiply.

Schedule (the reason this version is fast): the per-block preamble
(transposes, corner-weight chain, Fourier angle) for block b+1 is emitted in
the middle of block b's hidden-layer phase, so the PE never drains between
blocks and the HAM clock governor stays at full rate. Input DMAs are issued
two blocks ahead and before the (much larger) weight DMAs so compute starts
~6us into the kernel instead of ~42us. Activations live in [feature, sample]
layout; matmuls run as fp32r (full PE rate at N=512).
"""

import os
import numpy as np

N_CORES = 8
N_TOTAL = 65536
S_CORE = N_TOTAL // N_CORES          # 8192 samples per core
BLK = 512                            # samples per block
N_BLOCKS = S_CORE // BLK             # 16
EPS = 1e-5
N_LAYERS = 8                         # LN+relu layers (layer0 + 7 hidden)


def _precompute(inputs):
    """Host-side weight folding. Returns dict of constant arrays (fp32)."""
    convT_w = np.asarray(inputs["convT_w"], np.float32)
    W0 = np.asarray(inputs["W0"], np.float32)
    Wh = np.asarray(inputs["Wh"], np.float32)
    ln_g = np.asarray(inputs["ln_g"], np.float32)
    gauss = np.asarray(inputs["gauss"], np.float32)
    W_out = np.asarray(inputs["W_out"], np.float32)

    # A_stack[m*256+i, c] = convT_w[i, c, di, dj, dk], m = 4*di + 2*dj + dk
    A_stack = convT_w.transpose(2, 3, 4, 0, 1).reshape(8 * 256, 512)
    M0 = np.concatenate([A_stack @ W0[:512], W0[512:640], W0[640:768]], axis=0)

    def center_scale(W, g):
        Wc = W - W.mean(axis=1, keepdims=True)
        return np.ascontiguousarray(Wc * g[None, :], np.float32)

    W_eff = [center_scale(M0, ln_g[0])] + [
        center_scale(Wh[l], ln_g[l + 1]) for l in range(7)
    ]

    # pack each layer's weights as [128, n_kchunks, 512]
    def pack(W):
        K = W.shape[0]
        kc = K // 128
        return W.reshape(kc, 128, 512).transpose(1, 0, 2).reshape(128, kc * 512)

    w0p = np.ascontiguousarray(pack(W_eff[0]))                       # [128, 18*512]
    whp = np.ascontiguousarray(
        np.concatenate([pack(W) for W in W_eff[1:]], axis=1))        # [128, 28*512]
    # stats lhsT (layer 7 only, M=4): col (mc*4 + m) = 1/(512*g7[mc*128+k]^2)
    swv = (1.0 / (512.0 * ln_g[7] ** 2)).astype(np.float32)
    sw4 = np.empty((128, 16), np.float32)
    for mc in range(4):
        for m in range(4):
            sw4[:, mc * 4 + m] = swv[mc * 128:(mc + 1) * 128]

    return {
        "w0p": w0p,
        "whp": whp,
        "swp": np.ascontiguousarray(sw4),
        "ident": np.eye(128, dtype=np.float32),
        "gaussT": np.ascontiguousarray(gauss.T.astype(np.float32)),  # [3, 128]
        "sel128": _sel128(),
        "woutp": np.ascontiguousarray(
            W_out.reshape(4, 128, 4).transpose(1, 0, 2).reshape(128, 16)),
    }


def _sel128():
    # one-hot row selectors: sel128[k, m*128 + j] = (k == m), so a K=128
    # matmul broadcasts w8T row m to all 128 output partitions at full rate
    t = np.zeros((128, 8 * 128), np.float32)
    for m in range(8):
        t[m, m * 128:(m + 1) * 128] = 1.0
    return np.ascontiguousarray(t)


def _general_case_needed(inputs):
    z = lambda a: bool(np.all(np.asarray(a) == 0))
    return not (
        z(inputs["convT_b"]) and z(inputs["b0"]) and z(inputs["bh"])
        and z(inputs["ln_b"]) and z(inputs["b_out"])
        and bool(np.all(np.abs(np.asarray(inputs["ln_g"])) > 1e-3))
    )


def _numpy_fallback(inputs):
    """Reference in numpy (slow; only for inputs outside the fast path)."""
    inp = np.asarray(inputs["input"], np.float32)
    convT_w = np.asarray(inputs["convT_w"], np.float32)
    convT_b = np.asarray(inputs["convT_b"], np.float32)
    gauss = np.asarray(inputs["gauss"], np.float32)
    W0 = np.asarray(inputs["W0"], np.float32)
    b0 = np.asarray(inputs["b0"], np.float32)
    Wh = np.asarray(inputs["Wh"], np.float32)
    bh = np.asarray(inputs["bh"], np.float32)
    ln_g = np.asarray(inputs["ln_g"], np.float32)
    ln_b = np.asarray(inputs["ln_b"], np.float32)
    W_out = np.asarray(inputs["W_out"], np.float32)
    b_out = np.asarray(inputs["b_out"], np.float32)
    xyz = inp[:, -3:]
    lat = inp[:, :-3]
    f = (xyz + 1.0) * 0.5
    frac = f - np.clip(f.astype(np.int32), 0, 0)
    A = convT_w.transpose(2, 3, 4, 0, 1)
    lat_i = np.zeros((inp.shape[0], 512), np.float32)
    wx = [1 - frac[:, 0], frac[:, 0]]
    wy = [1 - frac[:, 1], frac[:, 1]]
    wz = [1 - frac[:, 2], frac[:, 2]]
    for di in (0, 1):
        for dj in (0, 1):
            for dk in (0, 1):
                w = (wx[di] * wy[dj] * wz[dk]).astype(np.float32)
                lat_i += (lat @ A[di, dj, dk]) * w[:, None]
    lat_i += convT_b[None, :]
    ang = 2 * np.pi * (xyz @ gauss.T)
    x = np.concatenate([lat_i, np.sin(ang), np.cos(ang)], axis=1)

    def ln(t, g, b):
        mu = t.mean(-1, keepdims=True)
        var = ((t - mu) ** 2).mean(-1, keepdims=True)
        return (t - mu) / np.sqrt(var + EPS) * g + b

    x = np.maximum(ln(x @ W0 + b0, ln_g[0], ln_b[0]), 0)
    for l in range(7):
        x = np.maximum(ln(x @ Wh[l] + bh[l], ln_g[l + 1], ln_b[l + 1]), 0)
    y = x @ W_out + b_out
    return np.concatenate([np.tanh(y[:, :1]), y[:, 1:] * 255.0], axis=1).astype(np.float32)


_NC_CACHE = {}


def _build_bass(s_core=S_CORE):
    """Build the per-core Bass module (SPMD; same program on all 8 cores)."""
    import concourse.bass as bass
    import concourse.bacc as bacc
    import concourse.tile as tile
    from concourse import mybir

    FP32 = mybir.dt.float32
    FP32R = mybir.dt.float32r
    I32 = mybir.dt.int32
    AF = mybir.ActivationFunctionType
    ALU = mybir.AluOpType
    TWO_PI = float(2.0 * np.pi)
    MAGIC = 12582912.0            # 1.5 * 2^23: fp32 add/sub rounds to integer
    RSQRT_SEED = 0x5F3759DF       # fp32 fast-inverse-sqrt seed constant
    n_blocks = s_core // BLK

    nc = bacc.Bacc("TRN2", target_bir_lowering=False, debug=False)

    inp_d = nc.dram_tensor("inp", [s_core, 259], FP32, kind="ExternalInput").ap()
    w0p_d = nc.dram_tensor("w0p", [128, 18 * 512], FP32R, kind="ExternalInput").ap()
    whp_d = nc.dram_tensor("whp", [128, 28 * 512], FP32R, kind="ExternalInput").ap()
    swp_d = nc.dram_tensor("swp", [128, 16], FP32R, kind="ExternalInput").ap()
    ident_d = nc.dram_tensor("ident", [128, 128], FP32R, kind="ExternalInput").ap()
    sel128_d = nc.dram_tensor("sel128", [128, 8 * 128], FP32R, kind="ExternalInput").ap()
    zeros_d = nc.dram_tensor("zeros512", [128, BLK], FP32R, kind="ExternalInput").ap()
    gaussT_d = nc.dram_tensor("gaussT", [3, 128], FP32R, kind="ExternalInput").ap()
    woutp_d = nc.dram_tensor("woutp", [128, 16], FP32R, kind="ExternalInput").ap()
    outT_d = nc.dram_tensor("outT", [4, s_core], FP32, kind="ExternalOutput").ap()

    def r(ap):
        return ap

    with tile.TileContext(nc) as tc:
        with (
            tc.tile_pool(name="const", bufs=1) as constp,
            tc.tile_pool(name="weights", bufs=1) as weightp,
            tc.tile_pool(name="inblk", bufs=2) as inp_pool,
            tc.tile_pool(name="acts", bufs=2) as actp,
            tc.tile_pool(name="scratch", bufs=2) as scr,
            tc.tile_pool(name="ps_t", bufs=1, space="PSUM") as ps_t,
            tc.tile_pool(name="ps_misc", bufs=2, space="PSUM") as ps_misc,
            tc.tile_pool(name="ps_gi", bufs=2, space="PSUM") as ps_gi,
        ):
            # ---- small constants first (so they beat the weight DMAs) ----
            ident_dma = constp.tile([128, 128], FP32R, name="ident_dma")
            nc.sync.dma_start(out=ident_dma, in_=ident_d)
            # DVE-gate the identity so PE transposes only ever wait on DVE
            ident_sb = constp.tile([128, 128], FP32R, name="ident_sb")
            nc.vector.tensor_copy(ident_sb, ident_dma)
            gaussT_sb = constp.tile([3, 128], FP32R)
            nc.sync.dma_start(out=gaussT_sb, in_=gaussT_d)
            sel128_sb = constp.tile([128, 8, 128], FP32R)
            nc.sync.dma_start(
                out=sel128_sb, in_=sel128_d.rearrange("p (m f) -> p m f", m=8))
            # persistent zero-padded corner-weight tile: rows 0-7 are
            # rewritten per block, rows 8-127 stay zero (selector weights
            # there are zero too, but 0*garbage could be NaN)
            w8tp = constp.tile([128, BLK], FP32R, name="w8tp")
            nc.sync.dma_start(out=w8tp, in_=zeros_d)
            wout_sb = weightp.tile([128, 4, 4], FP32R)
            nc.sync.dma_start(out=wout_sb, in_=woutp_d.rearrange("p (c f) -> p c f", c=4))
            sw_sb = weightp.tile([128, 16], FP32R)
            nc.sync.dma_start(out=sw_sb, in_=swp_d)

            inp_r = inp_d.rearrange("(b sc p) f -> b p sc f", sc=4, p=128)

            def load_start(b):
                """Issue the input DMA for block b (returns the landing tile)."""
                inb0 = inp_pool.tile([128, 4, 259], FP32, tag="inb0", name="inb0")
                nc.sync.dma_start(out=inb0, in_=inp_r[b])
                return inb0

            # prefetch the first two input blocks before the weight DMAs
            inb0_tiles = {0: load_start(0), 1: load_start(1)}

            # ---- weights, in consumption order ----
            w0_sb = weightp.tile([128, 18, 512], FP32R)
            w0r = w0p_d.rearrange("p (c f) -> p c f", c=18)
            for ch in range(3):
                nc.sync.dma_start(
                    out=w0_sb[:, ch * 6:(ch + 1) * 6, :], in_=w0r[:, ch * 6:(ch + 1) * 6, :])
            wh_sb = weightp.tile([128, 28, 512], FP32R)
            whr = whp_d.rearrange("p (c f) -> p c f", c=28)
            for ch in range(4):
                nc.sync.dma_start(
                    out=wh_sb[:, ch * 7:(ch + 1) * 7, :], in_=whr[:, ch * 7:(ch + 1) * 7, :])

            def preamble(b):
                """Emit block b's input-side prep. Called during block b-1's
                hidden phase (or standalone for b == 0): DVE corner-weight
                chain, PE transposes (fp32r, 1.5 c/r), Fourier angle + range
                reduction, and the sin/cos feature tiles."""
                inb = inp_pool.tile([128, 4, 259], FP32R, tag="inb", name="inb")
                nc.vector.tensor_copy(inb, inb0_tiles.pop(b))

                # corner weights in sample layout (DVE only; needs just inb)
                f3 = scr.tile([128, 4, 3], FP32R, tag="f3")
                nc.vector.tensor_scalar(
                    out=f3, in0=inb[:, :, 256:259],
                    scalar1=0.5, scalar2=0.5, op0=ALU.mult, op1=ALU.add,
                )
                om3 = scr.tile([128, 4, 3], FP32R, tag="om3")
                nc.vector.tensor_scalar(
                    out=om3, in0=f3, scalar1=1.0, scalar2=-1.0,
                    op0=ALU.subtract, op1=ALU.mult,
                )
                wxy = scr.tile([128, 4, 4], FP32R, tag="wxy")
                w8s = scr.tile([128, 4, 8], FP32R, tag="w8s")
                for di in (0, 1):
                    xs = (f3 if di else om3)[:, :, 0:1]
                    for dj in (0, 1):
                        ys = (f3 if dj else om3)[:, :, 1:2]
                        nc.vector.tensor_tensor(
                            out=wxy[:, :, di * 2 + dj:di * 2 + dj + 1],
                            in0=xs, in1=ys, op=ALU.mult,
                        )
                for m in range(8):
                    di, dj, dk = (m >> 2) & 1, (m >> 1) & 1, m & 1
                    zsl = (f3 if dk else om3)[:, :, 2:3]
                    nc.vector.tensor_tensor(
                        out=w8s[:, :, m:m + 1],
                        in0=wxy[:, :, di * 2 + dj:di * 2 + dj + 1],
                        in1=zsl, op=ALU.mult,
                    )

                # transposes to [feature, sample] (fp32r: 1.5 cycles/row)
                latT = scr.tile([128, 2, BLK], FP32R, tag="latT", bufs=2)
                xyzT = scr.tile([3, BLK], FP32R, tag="xyzT", bufs=2)
                for sc in range(4):
                    for fc in range(2):
                        tp = ps_misc.tile([128, 128], FP32R, tag="mt")
                        nc.tensor.transpose(
                            tp, inb[:, sc, fc * 128:(fc + 1) * 128], ident_sb
                        )
                        nc.vector.tensor_copy(latT[:, fc, sc * 128:(sc + 1) * 128], tp)
                    tp3 = ps_misc.tile([3, 128], FP32R, tag="mt")
                    nc.tensor.transpose(tp3, inb[:, sc, 256:259], ident_sb)
                    nc.vector.tensor_copy(xyzT[:, sc * 128:(sc + 1) * 128], tp3)
                    tp8 = ps_misc.tile([8, 128], FP32R, tag="mt")
                    nc.tensor.transpose(tp8, w8s[:, sc, :], ident_sb)
                    nc.vector.tensor_copy(w8tp[0:8, sc * 128:(sc + 1) * 128], tp8)

                # fourier angle, range-reduced to [-0.5, 0.5]
                angp = ps_misc.tile([128, BLK], FP32, tag="mt")
                nc.tensor.matmul(angp, r(gaussT_sb), r(xyzT), start=True, stop=True)
                ang_sb = scr.tile([128, BLK], FP32, tag="rr", bufs=3, name="ang_sb")
                nc.vector.tensor_copy(ang_sb, angp)
                zs_r = scr.tile([128, BLK], FP32, tag="rr", bufs=3, name="zs_r")
                nc.vector.tensor_scalar(
                    out=zs_r, in0=ang_sb, scalar1=MAGIC, scalar2=MAGIC,
                    op0=ALU.add, op1=ALU.subtract,
                )
                zs = scr.tile([128, BLK], FP32, tag="zs", bufs=1)
                nc.vector.tensor_sub(zs, ang_sb, zs_r)
                a25 = scr.tile([128, BLK], FP32, tag="a25", bufs=1)
                nc.vector.tensor_scalar_add(out=a25, in0=ang_sb, scalar1=0.25)
                zc_r = scr.tile([128, BLK], FP32, tag="rr", bufs=3, name="zc_r")
                nc.vector.tensor_scalar(
                    out=zc_r, in0=a25, scalar1=MAGIC, scalar2=MAGIC,
                    op0=ALU.add, op1=ALU.subtract,
                )
                zc = scr.tile([128, BLK], FP32, tag="zc", bufs=1)
                nc.vector.tensor_sub(zc, a25, zc_r)
                ffs = scr.tile([128, BLK], FP32R, tag="ff", bufs=2, name="ffs")
                nc.scalar.activation(out=ffs, in_=zs, func=AF.Sin, scale=TWO_PI)
                ffc = scr.tile([128, BLK], FP32R, tag="ff", bufs=2, name="ffc")
                nc.scalar.activation(out=ffc, in_=zc, func=AF.Sin, scale=TWO_PI)
                return latT, ffs, ffc

            def make_finalize(yp, gp, b):
                """Deferred block finalize: rg = 1/sqrt(gp) via the fp32
                bit-trick seed + 2 Newton steps (all DVE; keeps the ACT
                engine on a single activation table for the whole kernel),
                then yv = 255 * yp * rg, tanh on row 0, and the output DMA.
                Emitted a few uch-products into the NEXT block's layer 0 so
                it never head-of-line-blocks that block's DVE feed."""
                def fin():
                    sd = scr.tile([4, BLK], FP32, tag="nr_sd", bufs=1)
                    t1 = scr.tile([4, BLK], FP32, tag="nr_t1", bufs=1)
                    t2 = scr.tile([4, BLK], FP32, tag="nr_t2", bufs=1)
                    y1 = scr.tile([4, BLK], FP32, tag="nr_y1", bufs=1)
                    yv = scr.tile([4, BLK], FP32, tag="yv", bufs=1)
                    # seed bits = RSQRT_SEED - (bits(gp) >> 1)
                    nc.vector.tensor_scalar(
                        out=t1[0:4, :].bitcast(I32), in0=gp[0:4, :].bitcast(I32),
                        scalar1=1, scalar2=None, op0=ALU.logical_shift_right,
                    )
                    nc.vector.tensor_scalar(
                        out=sd[0:4, :].bitcast(I32), in0=t1[0:4, :].bitcast(I32),
                        scalar1=RSQRT_SEED, scalar2=-1,
                        op0=ALU.subtract, op1=ALU.mult,
                    )
                    # Newton 1: y1 = sd * (1.5 - 0.5 * gp * sd^2)
                    nc.vector.tensor_tensor(out=t1, in0=sd, in1=sd, op=ALU.mult)
                    nc.vector.scalar_tensor_tensor(
                        out=t2, in0=t1, scalar=-0.5, in1=gp[0:4, :],
                        op0=ALU.mult, op1=ALU.mult,
                    )
                    nc.vector.tensor_scalar_add(out=t2, in0=t2, scalar1=1.5)
                    nc.vector.tensor_tensor(out=y1, in0=sd, in1=t2, op=ALU.mult)
                    # Newton 2: rg = y1 * (1.5 - 0.5 * gp * y1^2)
                    nc.vector.tensor_tensor(out=t1, in0=y1, in1=y1, op=ALU.mult)
                    nc.vector.scalar_tensor_tensor(
                        out=t2, in0=t1, scalar=-0.5, in1=gp[0:4, :],
                        op0=ALU.mult, op1=ALU.mult,
                    )
                    nc.vector.tensor_scalar_add(out=t2, in0=t2, scalar1=1.5)
                    nc.vector.tensor_tensor(out=t1, in0=y1, in1=t2, op=ALU.mult)
                    # yv = (yp * 255) * rg ; out row0 = tanh(yv0 / 255)
                    nc.vector.scalar_tensor_tensor(
                        out=yv, in0=yp, scalar=255.0, in1=t1,
                        op0=ALU.mult, op1=ALU.mult,
                    )
                    nc.scalar.activation(
                        out=yv[0:1, :], in_=yv[0:1, :], func=AF.Tanh,
                        scale=1.0 / 255.0,
                    )
                    nc.sync.dma_start(
                        out=outT_d[:, b * BLK:(b + 1) * BLK], in_=yv)
                return fin

            pre_next = preamble(0)
            fin_prev = None

            for b in range(n_blocks):
                latT, ffs, ffc = pre_next
                if b + 2 < n_blocks:
                    inb0_tiles[b + 2] = load_start(b + 2)

                # ---- layer 0: build u chunks incrementally + matmul ----
                psums = [ps_t.tile([128, BLK], FP32, tag=f"pt{mc}", name=f"pt{mc}")
                         for mc in range(4)]
                uch_i = 0

                def l0_accum(u_ap, last=False):
                    nonlocal uch_i
                    for mc in range(4):
                        nc.tensor.matmul(
                            psums[mc],
                            r(w0_sb[:, uch_i, mc * 128:(mc + 1) * 128]),
                            r(u_ap),
                            start=(uch_i == 0), stop=last,
                        )
                    uch_i += 1

                for m in range(8):
                    bc = ps_misc.tile([128, BLK], FP32, tag="mt")
                    nc.tensor.matmul(
                        bc, r(sel128_sb[:, m, :]), r(w8tp), start=True, stop=True
                    )
                    for kc in range(2):
                        uch = scr.tile([128, BLK], FP32R, tag="uch", bufs=6)
                        nc.vector.tensor_tensor(
                            out=uch, in0=latT[:, kc, :], in1=bc, op=ALU.mult
                        )
                        l0_accum(uch)
                l0_accum(ffs)
                l0_accum(ffc, last=True)

                # ---- hidden LN+relu layers; block b+1's preamble is emitted
                # after layer 2 so every engine stream stays deep ----
                x_cur = None
                sq = None
                for j in range(N_LAYERS):
                    if j > 0:
                        psums = [
                            ps_t.tile([128, BLK], FP32, tag=f"pt{mc}", name=f"pt{mc}")
                            for mc in range(4)
                        ]
                        for mc in range(4):
                            for kc in range(4):
                                nc.tensor.matmul(
                                    psums[mc],
                                    r(wh_sb[:, (j - 1) * 4 + kc,
                                            mc * 128:(mc + 1) * 128]),
                                    r(x_cur[:, kc, :]),
                                    start=(kc == 0), stop=(kc == 3),
                                )
                    x_next = actp.tile([128, 4, BLK], FP32R, tag="xn")
                    for mc in range(4):
                        nc.scalar.activation(
                            out=x_next[:, mc, :], in_=psums[mc], func=AF.Relu
                        )
                    if j == 7:
                        sq = scr.tile([128, 4, BLK], FP32R, tag="sq", bufs=1)
                        for mc in range(4):
                            nc.scalar.activation(
                                out=sq[:, mc, :], in_=psums[mc], func=AF.Square
                            )
                    x_cur = x_next
                    if j == 1 and b + 1 < n_blocks:
                        pre_next = preamble(b + 1)
                    if j == 5 and fin_prev is not None:
                        fin_prev()
                        fin_prev = None

                # ---- output layer, then deferred-LN stats (sq is ready by
                # the time the wout matmuls finish) ----
                yp = ps_gi.tile([4, BLK], FP32, tag="gp", name="yp")
                for kc in range(4):
                    nc.tensor.matmul(
                        yp, r(wout_sb[:, kc, :]), r(x_cur[:, kc, :]),
                        start=(kc == 0), stop=(kc == 3),
                    )
                gp = ps_gi.tile([4, BLK], FP32, tag="gp", name="gp7")
                for mc in range(4):
                    nc.tensor.matmul(
                        gp, r(sw_sb[:, mc * 4:(mc + 1) * 4]),
                        r(sq[:, mc, :]), start=(mc == 0), stop=(mc == 3),
                    )
                fin_prev = make_finalize(yp, gp, b)

            fin_prev()

    nc.compile()
    return nc


def kernel(**inputs):
    if _general_case_needed(inputs):
        return _numpy_fallback(inputs)

    from concourse.bass_utils import run_bass_kernel_spmd

    pre = _precompute(inputs)
    inp = np.ascontiguousarray(np.asarray(inputs["input"], np.float32))

    if "nc" not in _NC_CACHE:
        _NC_CACHE["nc"] = _build_bass()
    nc = _NC_CACHE["nc"]

    in_maps = [
        {
            "inp": np.ascontiguousarray(inp[c * S_CORE:(c + 1) * S_CORE]),
            "w0p": pre["w0p"], "whp": pre["whp"], "swp": pre["swp"],
            "ident": pre["ident"], "gaussT": pre["gaussT"],
            "sel128": pre["sel128"], "woutp": pre["woutp"],
            "zeros512": np.zeros((128, BLK), np.float32),
        }
        for c in range(N_CORES)
    ]

    res = run_bass_kernel_spmd(
        nc, in_maps, core_ids=list(range(N_CORES)),
        trace=bool(int(os.environ.get("KERNEL_TRACE", "0"))),
    )
    kernel.last_results = res
    outs = [res.results[c]["outT"] for c in range(N_CORES)]
    return np.ascontiguousarray(
        np.concatenate([o.T for o in outs], axis=0).astype(np.float32)
    )


# revision 22
# speedup vs baseline: 1.2974x; 1.0471x over previous
"""Trainium2 Bass kernel for nn_Decoder (latent-grid decoder MLP).

Contract: kernel(**inputs) takes the FULL unsharded inputs (as produced by
setup_inputs()) and returns the FULL [65536, 4] float32 output. Internally the
65536 points are sharded across 8 NeuronCores (pure data parallel); the small
weights are replicated.

Algorithm (mathematically equivalent to the reference):
  - G=2 trilinear interp of a per-sample 2x2x2 grid always lands in cell
    (0,0,0) (indices clip to [0, G-2] = [0,0]), so
    lat_i = sum_m w_m(xyz) * (lat @ A_m), A_m = convT_w[:, :, di, dj, dk].
  - The interp + Fourier features + first MLP layer fold into one matmul:
    u = [w_0*lat, ..., w_7*lat, sin(2 pi ang), cos(2 pi ang)]  (2304 dims),
    h0 = u @ M0 with M0 = [A_stack @ W0_top; W0_sin; W0_cos] (host-folded).
  - LayerNorm mean-subtraction folds into the weights (column centering);
    ln gamma folds in too. The per-sample rstd is deferred via LN's positive
    scale invariance: activations stay unnormalized, and the final scale is
    1/sqrt(gi2) with gi2 = ssq(h7)/512 (the eps*gi2_6 correction term is
    ~1e-4 relative and is dropped; a tiny bias guards ssq == 0).
    Requires all biases and ln_b == 0 (true for this model; a numpy fallback
    covers the general case).
  - ssq(h7) is accumulated by the TensorEngine itself (weighted-ones
    stationary operand, M=4) into a [4, S] PSUM tile; the host folds a
    constant c into those weights so the Dsqrt activation-table input is
    centered near 0.25, and 255/sqrt(.) comes out of one ACT Dsqrt op
    (Dsqrt(x) = 1/(2 sqrt(x))) plus one DVE multiply.

Schedule (the reason this version is fast): the per-block preamble
(transposes, corner-weight chain, Fourier angle) for block b+1 is emitted in
the middle of block b's hidden-layer phase, so the PE never drains between
blocks and the HAM clock governor stays at full rate. Input DMAs are issued
two blocks ahead and before the (much larger) weight DMAs so compute starts
~6us into the kernel instead of ~42us. Activations live in [feature, sample]
layout; matmuls run as fp32r (full PE rate at N=512).
"""

import os
import ml_dtypes
import numpy as np

N_CORES = 8
N_TOTAL = 65536
S_CORE = N_TOTAL // N_CORES          # 8192 samples per core
BLK = 512                            # samples per block
N_BLOCKS = S_CORE // BLK             # 16
EPS = 1e-5
N_LAYERS = 8                         # LN+relu layers (layer0 + 7 hidden)


def _precompute(inputs):
    """Host-side weight folding. Returns dict of constant arrays (fp32)."""
    convT_w = np.asarray(inputs["convT_w"], np.float32)
    W0 = np.asarray(inputs["W0"], np.float32)
    Wh = np.asarray(inputs["Wh"], np.float32)
    ln_g = np.asarray(inputs["ln_g"], np.float32)
    gauss = np.asarray(inputs["gauss"], np.float32)
    W_out = np.asarray(inputs["W_out"], np.float32)

    # A_stack[m*256+i, c] = convT_w[i, c, di, dj, dk], m = 4*di + 2*dj + dk
    A_stack = convT_w.transpose(2, 3, 4, 0, 1).reshape(8 * 256, 512)
    M0 = np.concatenate([A_stack @ W0[:512], W0[512:640], W0[640:768]], axis=0)

    def center_scale(W, g):
        Wc = W - W.mean(axis=1, keepdims=True)
        return np.ascontiguousarray(Wc * g[None, :], np.float32)

    W_eff = [center_scale(M0, ln_g[0])] + [
        center_scale(Wh[l], ln_g[l + 1]) for l in range(7)
    ]

    # pack each layer's weights as [128, n_kchunks, 512]
    def pack(W):
        K = W.shape[0]
        kc = K // 128
        return W.reshape(kc, 128, 512).transpose(1, 0, 2).reshape(128, kc * 512)

    # main-path weights in bf16: halves the startup weight DMA and doubles
    # LDWEIGHTS rate (FWL); adds ~0.1% relative error per layer
    w0p = np.ascontiguousarray(pack(W_eff[0]).astype(ml_dtypes.bfloat16))
    whp = np.ascontiguousarray(
        np.concatenate([pack(W) for W in W_eff[1:]], axis=1)
        .astype(ml_dtypes.bfloat16))                                 # [128, 28*512]
    # stats lhsT (layer 7 only, M=4): col (mc*4 + m) = 1/(512*g7[mc*128+k]^2)
    swv = (1.0 / (512.0 * ln_g[7] ** 2)).astype(np.float32)
    sw4 = np.empty((128, 16), np.float32)
    for mc in range(4):
        for m in range(4):
            sw4[:, mc * 4 + m] = swv[mc * 128:(mc + 1) * 128]

    return {
        "w0p": w0p,
        "whp": whp,
        "swp": np.ascontiguousarray(sw4.astype(ml_dtypes.bfloat16)),
        "ident": np.eye(128, dtype=np.float32),
        "gaussT": np.ascontiguousarray(gauss.T.astype(np.float32)),  # [3, 128]
        "sel128": _sel128(),
        "woutp": np.ascontiguousarray(
            W_out.reshape(4, 128, 4).transpose(1, 0, 2).reshape(128, 16)
            .astype(ml_dtypes.bfloat16)),
    }


def _sel128():
    # one-hot row selectors: sel128[k, m*128 + j] = (k == m), so a K=128
    # matmul broadcasts w8T row m to all 128 output partitions at full rate
    t = np.zeros((128, 8 * 128), np.float32)
    for m in range(8):
        t[m, m * 128:(m + 1) * 128] = 1.0
    return np.ascontiguousarray(t)


def _general_case_needed(inputs):
    z = lambda a: bool(np.all(np.asarray(a) == 0))
    return not (
        z(inputs["convT_b"]) and z(inputs["b0"]) and z(inputs["bh"])
        and z(inputs["ln_b"]) and z(inputs["b_out"])
        and bool(np.all(np.abs(np.asarray(inputs["ln_g"])) > 1e-3))
    )


def _numpy_fallback(inputs):
    """Reference in numpy (slow; only for inputs outside the fast path)."""
    inp = np.asarray(inputs["input"], np.float32)
    convT_w = np.asarray(inputs["convT_w"], np.float32)
    convT_b = np.asarray(inputs["convT_b"], np.float32)
    gauss = np.asarray(inputs["gauss"], np.float32)
    W0 = np.asarray(inputs["W0"], np.float32)
    b0 = np.asarray(inputs["b0"], np.float32)
    Wh = np.asarray(inputs["Wh"], np.float32)
    bh = np.asarray(inputs["bh"], np.float32)
    ln_g = np.asarray(inputs["ln_g"], np.float32)
    ln_b = np.asarray(inputs["ln_b"], np.float32)
    W_out = np.asarray(inputs["W_out"], np.float32)
    b_out = np.asarray(inputs["b_out"], np.float32)
    xyz = inp[:, -3:]
    lat = inp[:, :-3]
    f = (xyz + 1.0) * 0.5
    frac = f - np.clip(f.astype(np.int32), 0, 0)
    A = convT_w.transpose(2, 3, 4, 0, 1)
    lat_i = np.zeros((inp.shape[0], 512), np.float32)
    wx = [1 - frac[:, 0], frac[:, 0]]
    wy = [1 - frac[:, 1], frac[:, 1]]
    wz = [1 - frac[:, 2], frac[:, 2]]
    for di in (0, 1):
        for dj in (0, 1):
            for dk in (0, 1):
                w = (wx[di] * wy[dj] * wz[dk]).astype(np.float32)
                lat_i += (lat @ A[di, dj, dk]) * w[:, None]
    lat_i += convT_b[None, :]
    ang = 2 * np.pi * (xyz @ gauss.T)
    x = np.concatenate([lat_i, np.sin(ang), np.cos(ang)], axis=1)

    def ln(t, g, b):
        mu = t.mean(-1, keepdims=True)
        var = ((t - mu) ** 2).mean(-1, keepdims=True)
        return (t - mu) / np.sqrt(var + EPS) * g + b

    x = np.maximum(ln(x @ W0 + b0, ln_g[0], ln_b[0]), 0)
    for l in range(7):
        x = np.maximum(ln(x @ Wh[l] + bh[l], ln_g[l + 1], ln_b[l + 1]), 0)
    y = x @ W_out + b_out
    return np.concatenate([np.tanh(y[:, :1]), y[:, 1:] * 255.0], axis=1).astype(np.float32)


_NC_CACHE = {}


def _build_bass(s_core=S_CORE):
    """Build the per-core Bass module (SPMD; same program on all 8 cores)."""
    import concourse.bass as bass
    import concourse.bacc as bacc
    import concourse.tile as tile
    from concourse import mybir

    FP32 = mybir.dt.float32
    FP32R = mybir.dt.float32r
    BF16 = mybir.dt.bfloat16
    I32 = mybir.dt.int32
    AF = mybir.ActivationFunctionType
    ALU = mybir.AluOpType
    TWO_PI = float(2.0 * np.pi)
    MAGIC = 12582912.0            # 1.5 * 2^23: fp32 add/sub rounds to integer
    RSQRT_SEED = 0x5F3759DF       # fp32 fast-inverse-sqrt seed constant
    n_blocks = s_core // BLK

    nc = bacc.Bacc("TRN2", target_bir_lowering=False, debug=False)

    inp_d = nc.dram_tensor("inp", [s_core, 259], FP32, kind="ExternalInput").ap()
    w0p_d = nc.dram_tensor("w0p", [128, 18 * 512], BF16, kind="ExternalInput").ap()
    whp_d = nc.dram_tensor("whp", [128, 28 * 512], BF16, kind="ExternalInput").ap()
    swp_d = nc.dram_tensor("swp", [128, 16], BF16, kind="ExternalInput").ap()
    ident_d = nc.dram_tensor("ident", [128, 128], FP32R, kind="ExternalInput").ap()
    sel128_d = nc.dram_tensor("sel128", [128, 8 * 128], FP32R, kind="ExternalInput").ap()
    zeros_d = nc.dram_tensor("zeros512", [128, BLK], FP32R, kind="ExternalInput").ap()
    gaussT_d = nc.dram_tensor("gaussT", [3, 128], FP32R, kind="ExternalInput").ap()
    woutp_d = nc.dram_tensor("woutp", [128, 16], BF16, kind="ExternalInput").ap()
    outT_d = nc.dram_tensor("outT", [4, s_core], FP32, kind="ExternalOutput").ap()

    def r(ap):
        return ap

    with tile.TileContext(nc) as tc:
        with (
            tc.tile_pool(name="const", bufs=1) as constp,
            tc.tile_pool(name="weights", bufs=1) as weightp,
            tc.tile_pool(name="inblk", bufs=2) as inp_pool,
            tc.tile_pool(name="acts", bufs=2) as actp,
            tc.tile_pool(name="scratch", bufs=2) as scr,
            tc.tile_pool(name="ps_t", bufs=1, space="PSUM") as ps_t,
            tc.tile_pool(name="ps_misc", bufs=2, space="PSUM") as ps_misc,
            tc.tile_pool(name="ps_gi", bufs=2, space="PSUM") as ps_gi,
        ):
            # ---- small constants first (so they beat the weight DMAs) ----
            ident_dma = constp.tile([128, 128], FP32R, name="ident_dma")
            nc.sync.dma_start(out=ident_dma, in_=ident_d)
            # DVE-gate the identity so PE transposes only ever wait on DVE
            ident_sb = constp.tile([128, 128], FP32R, name="ident_sb")
            nc.vector.tensor_copy(ident_sb, ident_dma)
            gaussT_sb = constp.tile([3, 128], FP32R)
            nc.sync.dma_start(out=gaussT_sb, in_=gaussT_d)
            sel128_sb = constp.tile([128, 8, 128], FP32R)
            nc.sync.dma_start(
                out=sel128_sb, in_=sel128_d.rearrange("p (m f) -> p m f", m=8))
            # persistent zero-padded corner-weight tile: rows 0-7 are
            # rewritten per block, rows 8-127 stay zero (selector weights
            # there are zero too, but 0*garbage could be NaN)
            w8tp = constp.tile([128, BLK], FP32R, name="w8tp")
            nc.sync.dma_start(out=w8tp, in_=zeros_d)
            wout_sb = weightp.tile([128, 4, 4], BF16)
            nc.sync.dma_start(out=wout_sb, in_=woutp_d.rearrange("p (c f) -> p c f", c=4))
            sw_sb = weightp.tile([128, 16], BF16)
            nc.sync.dma_start(out=sw_sb, in_=swp_d)

            inp_r = inp_d.rearrange("(b sc p) f -> b p sc f", sc=4, p=128)

            def load_start(b):
                """Issue the input DMA for block b (returns the landing tile)."""
                inb0 = inp_pool.tile([128, 4, 259], FP32, tag="inb0", name="inb0")
                nc.sync.dma_start(out=inb0, in_=inp_r[b])
                return inb0

            # prefetch the first two input blocks before the weight DMAs
            inb0_tiles = {0: load_start(0), 1: load_start(1)}

            # ---- weights, in consumption order ----
            w0_sb = weightp.tile([128, 18, 512], BF16)
            w0r = w0p_d.rearrange("p (c f) -> p c f", c=18)
            for ch in range(3):
                nc.sync.dma_start(
                    out=w0_sb[:, ch * 6:(ch + 1) * 6, :], in_=w0r[:, ch * 6:(ch + 1) * 6, :])
            wh_sb = weightp.tile([128, 28, 512], BF16)
            whr = whp_d.rearrange("p (c f) -> p c f", c=28)
            for ch in range(4):
                nc.sync.dma_start(
                    out=wh_sb[:, ch * 7:(ch + 1) * 7, :], in_=whr[:, ch * 7:(ch + 1) * 7, :])

            def preamble(b):
                """Emit block b's input-side prep. Called during block b-1's
                hidden phase (or standalone for b == 0): DVE corner-weight
                chain, PE transposes (fp32r, 1.5 c/r), Fourier angle + range
                reduction, and the sin/cos feature tiles."""
                inb = inp_pool.tile([128, 4, 259], FP32R, tag="inb", name="inb")
                nc.vector.tensor_copy(inb, inb0_tiles.pop(b))

                # corner weights in sample layout (DVE only; needs just inb)
                f3 = scr.tile([128, 4, 3], FP32R, tag="f3")
                nc.vector.tensor_scalar(
                    out=f3, in0=inb[:, :, 256:259],
                    scalar1=0.5, scalar2=0.5, op0=ALU.mult, op1=ALU.add,
                )
                om3 = scr.tile([128, 4, 3], FP32R, tag="om3")
                nc.vector.tensor_scalar(
                    out=om3, in0=f3, scalar1=1.0, scalar2=-1.0,
                    op0=ALU.subtract, op1=ALU.mult,
                )
                wxy = scr.tile([128, 4, 4], FP32R, tag="wxy")
                w8s = scr.tile([128, 4, 8], FP32R, tag="w8s")
                for di in (0, 1):
                    xs = (f3 if di else om3)[:, :, 0:1]
                    for dj in (0, 1):
                        ys = (f3 if dj else om3)[:, :, 1:2]
                        nc.vector.tensor_tensor(
                            out=wxy[:, :, di * 2 + dj:di * 2 + dj + 1],
                            in0=xs, in1=ys, op=ALU.mult,
                        )
                for m in range(8):
                    di, dj, dk = (m >> 2) & 1, (m >> 1) & 1, m & 1
                    zsl = (f3 if dk else om3)[:, :, 2:3]
                    nc.vector.tensor_tensor(
                        out=w8s[:, :, m:m + 1],
                        in0=wxy[:, :, di * 2 + dj:di * 2 + dj + 1],
                        in1=zsl, op=ALU.mult,
                    )

                # transposes to [feature, sample] (fp32r: 1.5 cycles/row)
                latT = scr.tile([128, 2, BLK], FP32R, tag="latT", bufs=2)
                xyzT = scr.tile([3, BLK], FP32R, tag="xyzT", bufs=2)
                for sc in range(4):
                    for fc in range(2):
                        tp = ps_misc.tile([128, 128], FP32R, tag="mt")
                        nc.tensor.transpose(
                            tp, inb[:, sc, fc * 128:(fc + 1) * 128], ident_sb
                        )
                        nc.vector.tensor_copy(latT[:, fc, sc * 128:(sc + 1) * 128], tp)
                    tp3 = ps_misc.tile([3, 128], FP32R, tag="mt")
                    nc.tensor.transpose(tp3, inb[:, sc, 256:259], ident_sb)
                    nc.vector.tensor_copy(xyzT[:, sc * 128:(sc + 1) * 128], tp3)
                    tp8 = ps_misc.tile([8, 128], FP32R, tag="mt")
                    nc.tensor.transpose(tp8, w8s[:, sc, :], ident_sb)
                    nc.vector.tensor_copy(w8tp[0:8, sc * 128:(sc + 1) * 128], tp8)

                # fourier angle, range-reduced to [-0.5, 0.5]
                angp = ps_misc.tile([128, BLK], FP32, tag="mt")
                nc.tensor.matmul(angp, r(gaussT_sb), r(xyzT), start=True, stop=True)
                ang_sb = scr.tile([128, BLK], FP32, tag="rr", bufs=3, name="ang_sb")
                nc.vector.tensor_copy(ang_sb, angp)
                zs_r = scr.tile([128, BLK], FP32, tag="rr", bufs=3, name="zs_r")
                nc.vector.tensor_scalar(
                    out=zs_r, in0=ang_sb, scalar1=MAGIC, scalar2=MAGIC,
                    op0=ALU.add, op1=ALU.subtract,
                )
                zs = scr.tile([128, BLK], FP32, tag="zs", bufs=1)
                nc.vector.tensor_sub(zs, ang_sb, zs_r)
                a25 = scr.tile([128, BLK], FP32, tag="a25", bufs=1)
                nc.vector.tensor_scalar_add(out=a25, in0=ang_sb, scalar1=0.25)
                zc_r = scr.tile([128, BLK], FP32, tag="rr", bufs=3, name="zc_r")
                nc.vector.tensor_scalar(
                    out=zc_r, in0=a25, scalar1=MAGIC, scalar2=MAGIC,
                    op0=ALU.add, op1=ALU.subtract,
                )
                zc = scr.tile([128, BLK], FP32, tag="zc", bufs=1)
                nc.vector.tensor_sub(zc, a25, zc_r)
                ffs = scr.tile([128, BLK], BF16, tag="ff", bufs=2, name="ffs")
                nc.scalar.activation(out=ffs, in_=zs, func=AF.Sin, scale=TWO_PI)
                ffc = scr.tile([128, BLK], BF16, tag="ff", bufs=2, name="ffc")
                nc.scalar.activation(out=ffc, in_=zc, func=AF.Sin, scale=TWO_PI)
                return latT, ffs, ffc

            def make_finalize(yp, gp, b):
                """Deferred block finalize: rg = 1/sqrt(gp) via the fp32
                bit-trick seed + 2 Newton steps (all DVE; keeps the ACT
                engine on a single activation table for the whole kernel),
                then yv = 255 * yp * rg, tanh on row 0, and the output DMA.
                Emitted a few uch-products into the NEXT block's layer 0 so
                it never head-of-line-blocks that block's DVE feed."""
                def fin():
                    sd = scr.tile([4, BLK], FP32, tag="nr_sd", bufs=1)
                    t1 = scr.tile([4, BLK], FP32, tag="nr_t1", bufs=1)
                    t2 = scr.tile([4, BLK], FP32, tag="nr_t2", bufs=1)
                    y1 = scr.tile([4, BLK], FP32, tag="nr_y1", bufs=1)
                    yv = scr.tile([4, BLK], FP32, tag="yv", bufs=1)
                    # seed bits = RSQRT_SEED - (bits(gp) >> 1)
                    nc.vector.tensor_scalar(
                        out=t1[0:4, :].bitcast(I32), in0=gp[0:4, :].bitcast(I32),
                        scalar1=1, scalar2=None, op0=ALU.logical_shift_right,
                    )
                    nc.vector.tensor_scalar(
                        out=sd[0:4, :].bitcast(I32), in0=t1[0:4, :].bitcast(I32),
                        scalar1=RSQRT_SEED, scalar2=-1,
                        op0=ALU.subtract, op1=ALU.mult,
                    )
                    # Newton 1: y1 = sd * (1.5 - 0.5 * gp * sd^2)
                    nc.vector.tensor_tensor(out=t1, in0=sd, in1=sd, op=ALU.mult)
                    nc.vector.scalar_tensor_tensor(
                        out=t2, in0=t1, scalar=-0.5, in1=gp[0:4, :],
                        op0=ALU.mult, op1=ALU.mult,
                    )
                    nc.vector.tensor_scalar_add(out=t2, in0=t2, scalar1=1.5)
                    nc.vector.tensor_tensor(out=y1, in0=sd, in1=t2, op=ALU.mult)
                    # Newton 2: rg = y1 * (1.5 - 0.5 * gp * y1^2)
                    nc.vector.tensor_tensor(out=t1, in0=y1, in1=y1, op=ALU.mult)
                    nc.vector.scalar_tensor_tensor(
                        out=t2, in0=t1, scalar=-0.5, in1=gp[0:4, :],
                        op0=ALU.mult, op1=ALU.mult,
                    )
                    nc.vector.tensor_scalar_add(out=t2, in0=t2, scalar1=1.5)
                    nc.vector.tensor_tensor(out=t1, in0=y1, in1=t2, op=ALU.mult)
                    # yv = (yp * 255) * rg ; out row0 = tanh(yv0 / 255)
                    nc.vector.scalar_tensor_tensor(
                        out=yv, in0=yp, scalar=255.0, in1=t1,
                        op0=ALU.mult, op1=ALU.mult,
                    )
                    nc.scalar.activation(
                        out=yv[0:1, :], in_=yv[0:1, :], func=AF.Tanh,
                        scale=1.0 / 255.0,
                    )
                    nc.sync.dma_start(
                        out=outT_d[:, b * BLK:(b + 1) * BLK], in_=yv)
                return fin

            pre_next = preamble(0)
            fin_prev = None

            for b in range(n_blocks):
                latT, ffs, ffc = pre_next
                if b + 2 < n_blocks:
                    inb0_tiles[b + 2] = load_start(b + 2)

                # ---- layer 0: build u chunks incrementally + matmul ----
                psums = [ps_t.tile([128, BLK], FP32, tag=f"pt{mc}", name=f"pt{mc}")
                         for mc in range(4)]
                uch_i = 0

                def l0_accum(u_ap, last=False):
                    nonlocal uch_i
                    for mc in range(4):
                        nc.tensor.matmul(
                            psums[mc],
                            r(w0_sb[:, uch_i, mc * 128:(mc + 1) * 128]),
                            r(u_ap),
                            start=(uch_i == 0), stop=last,
                        )
                    uch_i += 1

                for m in range(8):
                    bc = ps_misc.tile([128, BLK], FP32, tag="mt")
                    nc.tensor.matmul(
                        bc, r(sel128_sb[:, m, :]), r(w8tp), start=True, stop=True
                    )
                    for kc in range(2):
                        uch = scr.tile([128, BLK], BF16, tag="uch", bufs=6)
                        nc.vector.tensor_tensor(
                            out=uch, in0=latT[:, kc, :], in1=bc, op=ALU.mult
                        )
                        l0_accum(uch)
                l0_accum(ffs)
                l0_accum(ffc, last=True)

                # ---- hidden LN+relu layers; block b+1's preamble is emitted
                # after layer 2 so every engine stream stays deep ----
                x_cur = None
                sq = None
                for j in range(N_LAYERS):
                    if j > 0:
                        psums = [
                            ps_t.tile([128, BLK], FP32, tag=f"pt{mc}", name=f"pt{mc}")
                            for mc in range(4)
                        ]
                        for mc in range(4):
                            for kc in range(4):
                                nc.tensor.matmul(
                                    psums[mc],
                                    r(wh_sb[:, (j - 1) * 4 + kc,
                                            mc * 128:(mc + 1) * 128]),
                                    r(x_cur[:, kc, :]),
                                    start=(kc == 0), stop=(kc == 3),
                                )
                    x_next = actp.tile([128, 4, BLK], BF16, tag="xn")
                    for mc in range(4):
                        nc.scalar.activation(
                            out=x_next[:, mc, :], in_=psums[mc], func=AF.Relu
                        )
                    if j == 7:
                        sq = scr.tile([128, 4, BLK], BF16, tag="sq", bufs=1)
                        for mc in range(4):
                            nc.scalar.activation(
                                out=sq[:, mc, :], in_=psums[mc], func=AF.Square
                            )
                    x_cur = x_next
                    if j == 1 and b + 1 < n_blocks:
                        pre_next = preamble(b + 1)
                    if j == 5 and fin_prev is not None:
                        fin_prev()
                        fin_prev = None

                # ---- output layer, then deferred-LN stats (sq is ready by
                # the time the wout matmuls finish) ----
                yp = ps_gi.tile([4, BLK], FP32, tag="gp", name="yp")
                for kc in range(4):
                    nc.tensor.matmul(
                        yp, r(wout_sb[:, kc, :]), r(x_cur[:, kc, :]),
                        start=(kc == 0), stop=(kc == 3),
                    )
                gp = ps_gi.tile([4, BLK], FP32, tag="gp", name="gp7")
                for mc in range(4):
                    nc.tensor.matmul(
                        gp, r(sw_sb[:, mc * 4:(mc + 1) * 4]),
                        r(sq[:, mc, :]), start=(mc == 0), stop=(mc == 3),
                    )
                fin_prev = make_finalize(yp, gp, b)

            fin_prev()

    nc.compile()
    return nc


def kernel(**inputs):
    if _general_case_needed(inputs):
        return _numpy_fallback(inputs)

    from concourse.bass_utils import run_bass_kernel_spmd

    pre = _precompute(inputs)
    inp = np.ascontiguousarray(np.asarray(inputs["input"], np.float32))

    if "nc" not in _NC_CACHE:
        _NC_CACHE["nc"] = _build_bass()
    nc = _NC_CACHE["nc"]

    in_maps = [
        {
            "inp": np.ascontiguousarray(inp[c * S_CORE:(c + 1) * S_CORE]),
            "w0p": pre["w0p"], "whp": pre["whp"], "swp": pre["swp"],
            "ident": pre["ident"], "gaussT": pre["gaussT"],
            "sel128": pre["sel128"], "woutp": pre["woutp"],
            "zeros512": np.zeros((128, BLK), np.float32),
        }
        for c in range(N_CORES)
    ]

    res = run_bass_kernel_spmd(
        nc, in_maps, core_ids=list(range(N_CORES)),
        trace=bool(int(os.environ.get("KERNEL_TRACE", "0"))),
    )
    kernel.last_results = res
    outs = [res.results[c]["outT"] for c in range(N_CORES)]
    return np.ascontiguousarray(
        np.concatenate([o.T for o in outs], axis=0).astype(np.float32)
    )


# revision 23
# speedup vs baseline: 1.3363x; 1.0299x over previous
"""Trainium2 Bass kernel for nn_Decoder (latent-grid decoder MLP).

Contract: kernel(**inputs) takes the FULL unsharded inputs (as produced by
setup_inputs()) and returns the FULL [65536, 4] float32 output. Internally the
65536 points are sharded across 8 NeuronCores (pure data parallel); the small
weights are replicated.

Algorithm (mathematically equivalent to the reference):
  - G=2 trilinear interp of a per-sample 2x2x2 grid always lands in cell
    (0,0,0) (indices clip to [0, G-2] = [0,0]), so
    lat_i = sum_m w_m(xyz) * (lat @ A_m), A_m = convT_w[:, :, di, dj, dk].
  - The interp + Fourier features + first MLP layer fold into one matmul:
    u = [w_0*lat, ..., w_7*lat, sin(2 pi ang), cos(2 pi ang)]  (2304 dims),
    h0 = u @ M0 with M0 = [A_stack @ W0_top; W0_sin; W0_cos] (host-folded).
  - LayerNorm mean-subtraction folds into the weights (column centering);
    ln gamma folds in too. The per-sample rstd is deferred via LN's positive
    scale invariance: activations stay unnormalized, and the final scale is
    1/sqrt(gi2) with gi2 = ssq(h7)/512 (the eps*gi2_6 correction term is
    ~1e-4 relative and is dropped; a tiny bias guards ssq == 0).
    Requires all biases and ln_b == 0 (true for this model; a numpy fallback
    covers the general case).
  - ssq(h7) is accumulated by the TensorEngine itself (weighted-ones
    stationary operand, M=4) into a [4, S] PSUM tile; the host folds a
    constant c into those weights so the Dsqrt activation-table input is
    centered near 0.25, and 255/sqrt(.) comes out of one ACT Dsqrt op
    (Dsqrt(x) = 1/(2 sqrt(x))) plus one DVE multiply.

Schedule (the reason this version is fast): the per-block preamble
(transposes, corner-weight chain, Fourier angle) for block b+1 is emitted in
the middle of block b's hidden-layer phase, so the PE never drains between
blocks and the HAM clock governor stays at full rate. Input DMAs are issued
two blocks ahead and before the (much larger) weight DMAs so compute starts
~6us into the kernel instead of ~42us. Activations live in [feature, sample]
layout; matmuls run as fp32r (full PE rate at N=512).
"""

import os
import numpy as np

N_CORES = 8
N_TOTAL = 65536
S_CORE = N_TOTAL // N_CORES          # 8192 samples per core
BLK = 512                            # samples per block
N_BLOCKS = S_CORE // BLK             # 16
EPS = 1e-5
N_LAYERS = 8                         # LN+relu layers (layer0 + 7 hidden)


def _precompute(inputs):
    """Host-side weight folding. Returns dict of constant arrays (fp32)."""
    convT_w = np.asarray(inputs["convT_w"], np.float32)
    W0 = np.asarray(inputs["W0"], np.float32)
    Wh = np.asarray(inputs["Wh"], np.float32)
    ln_g = np.asarray(inputs["ln_g"], np.float32)
    gauss = np.asarray(inputs["gauss"], np.float32)
    W_out = np.asarray(inputs["W_out"], np.float32)

    # A_stack[m*256+i, c] = convT_w[i, c, di, dj, dk], m = 4*di + 2*dj + dk
    A_stack = convT_w.transpose(2, 3, 4, 0, 1).reshape(8 * 256, 512)
    M0 = np.concatenate([A_stack @ W0[:512], W0[512:640], W0[640:768]], axis=0)

    def center_scale(W, g):
        Wc = W - W.mean(axis=1, keepdims=True)
        return np.ascontiguousarray(Wc * g[None, :], np.float32)

    W_eff = [center_scale(M0, ln_g[0])] + [
        center_scale(Wh[l], ln_g[l + 1]) for l in range(7)
    ]

    # Fold a constant gain into each layer so the unnormalized activations
    # stay near fp16's sweet spot (the deferred-LN final scale divides any
    # global constant back out, so this is exact). Estimated on 256 samples.
    rng = np.random.default_rng(0)
    ns = 256
    lat_s = rng.random((ns, 256), np.float32)
    xyz_s = rng.random((ns, 3), np.float32)
    f = (xyz_s + 1.0) * 0.5
    wx = [1 - f[:, 0], f[:, 0]]
    wy = [1 - f[:, 1], f[:, 1]]
    wz = [1 - f[:, 2], f[:, 2]]
    u = np.empty((ns, 2304), np.float32)
    for m in range(8):
        di, dj, dk = (m >> 2) & 1, (m >> 1) & 1, m & 1
        w = (wx[di] * wy[dj] * wz[dk]).astype(np.float32)
        u[:, m * 256:(m + 1) * 256] = lat_s * w[:, None]
    ang_s = 2 * np.pi * (xyz_s @ gauss.T)
    u[:, 2048:2176] = np.sin(ang_s)
    u[:, 2176:2304] = np.cos(ang_s)
    x = u
    for l in range(8):
        h = np.maximum(x, 0.0) @ W_eff[l] if l else x @ W_eff[l]
        rms = float(np.sqrt(np.mean(h * h)))
        c = min(max(0.35 / max(rms, 1e-30), 0.25), 64.0)
        W_eff[l] = W_eff[l] * c
        h = h * c
        x = h

    # pack each layer's weights as [128, n_kchunks, 512]
    def pack(W):
        K = W.shape[0]
        kc = K // 128
        return W.reshape(kc, 128, 512).transpose(1, 0, 2).reshape(128, kc * 512)

    # main-path weights in bf16: halves the startup weight DMA and doubles
    # LDWEIGHTS rate (FWL); adds ~0.1% relative error per layer
    w0p = np.ascontiguousarray(pack(W_eff[0]).astype(np.float16))
    whp = np.ascontiguousarray(
        np.concatenate([pack(W) for W in W_eff[1:]], axis=1)
        .astype(np.float16))                                 # [128, 28*512]
    # stats lhsT (layer 7 only, M=4): col (mc*4 + m) = 1/(512*g7[mc*128+k]^2)
    swv = (1.0 / (512.0 * ln_g[7] ** 2)).astype(np.float32)
    sw4 = np.empty((128, 16), np.float32)
    for mc in range(4):
        for m in range(4):
            sw4[:, mc * 4 + m] = swv[mc * 128:(mc + 1) * 128]

    return {
        "w0p": w0p,
        "whp": whp,
        "swp": np.ascontiguousarray(sw4.astype(np.float16)),
        "ident": np.eye(128, dtype=np.float32),
        "gaussT": np.ascontiguousarray(gauss.T.astype(np.float32)),  # [3, 128]
        "sel128": _sel128(),
        "woutp": np.ascontiguousarray(
            W_out.reshape(4, 128, 4).transpose(1, 0, 2).reshape(128, 16)
            .astype(np.float16)),
    }


def _sel128():
    # one-hot row selectors: sel128[k, m*128 + j] = (k == m), so a K=128
    # matmul broadcasts w8T row m to all 128 output partitions at full rate
    t = np.zeros((128, 8 * 128), np.float32)
    for m in range(8):
        t[m, m * 128:(m + 1) * 128] = 1.0
    return np.ascontiguousarray(t)


def _general_case_needed(inputs):
    z = lambda a: bool(np.all(np.asarray(a) == 0))
    return not (
        z(inputs["convT_b"]) and z(inputs["b0"]) and z(inputs["bh"])
        and z(inputs["ln_b"]) and z(inputs["b_out"])
        and bool(np.all(np.abs(np.asarray(inputs["ln_g"])) > 1e-3))
    )


def _numpy_fallback(inputs):
    """Reference in numpy (slow; only for inputs outside the fast path)."""
    inp = np.asarray(inputs["input"], np.float32)
    convT_w = np.asarray(inputs["convT_w"], np.float32)
    convT_b = np.asarray(inputs["convT_b"], np.float32)
    gauss = np.asarray(inputs["gauss"], np.float32)
    W0 = np.asarray(inputs["W0"], np.float32)
    b0 = np.asarray(inputs["b0"], np.float32)
    Wh = np.asarray(inputs["Wh"], np.float32)
    bh = np.asarray(inputs["bh"], np.float32)
    ln_g = np.asarray(inputs["ln_g"], np.float32)
    ln_b = np.asarray(inputs["ln_b"], np.float32)
    W_out = np.asarray(inputs["W_out"], np.float32)
    b_out = np.asarray(inputs["b_out"], np.float32)
    xyz = inp[:, -3:]
    lat = inp[:, :-3]
    f = (xyz + 1.0) * 0.5
    frac = f - np.clip(f.astype(np.int32), 0, 0)
    A = convT_w.transpose(2, 3, 4, 0, 1)
    lat_i = np.zeros((inp.shape[0], 512), np.float32)
    wx = [1 - frac[:, 0], frac[:, 0]]
    wy = [1 - frac[:, 1], frac[:, 1]]
    wz = [1 - frac[:, 2], frac[:, 2]]
    for di in (0, 1):
        for dj in (0, 1):
            for dk in (0, 1):
                w = (wx[di] * wy[dj] * wz[dk]).astype(np.float32)
                lat_i += (lat @ A[di, dj, dk]) * w[:, None]
    lat_i += convT_b[None, :]
    ang = 2 * np.pi * (xyz @ gauss.T)
    x = np.concatenate([lat_i, np.sin(ang), np.cos(ang)], axis=1)

    def ln(t, g, b):
        mu = t.mean(-1, keepdims=True)
        var = ((t - mu) ** 2).mean(-1, keepdims=True)
        return (t - mu) / np.sqrt(var + EPS) * g + b

    x = np.maximum(ln(x @ W0 + b0, ln_g[0], ln_b[0]), 0)
    for l in range(7):
        x = np.maximum(ln(x @ Wh[l] + bh[l], ln_g[l + 1], ln_b[l + 1]), 0)
    y = x @ W_out + b_out
    return np.concatenate([np.tanh(y[:, :1]), y[:, 1:] * 255.0], axis=1).astype(np.float32)


_NC_CACHE = {}


def _build_bass(s_core=S_CORE):
    """Build the per-core Bass module (SPMD; same program on all 8 cores)."""
    import concourse.bass as bass
    import concourse.bacc as bacc
    import concourse.tile as tile
    from concourse import mybir

    FP32 = mybir.dt.float32
    FP32R = mybir.dt.float32r
    F16 = mybir.dt.float16
    I32 = mybir.dt.int32
    AF = mybir.ActivationFunctionType
    ALU = mybir.AluOpType
    TWO_PI = float(2.0 * np.pi)
    MAGIC = 12582912.0            # 1.5 * 2^23: fp32 add/sub rounds to integer
    RSQRT_SEED = 0x5F3759DF       # fp32 fast-inverse-sqrt seed constant
    n_blocks = s_core // BLK

    nc = bacc.Bacc("TRN2", target_bir_lowering=False, debug=False)

    inp_d = nc.dram_tensor("inp", [s_core, 259], FP32, kind="ExternalInput").ap()
    w0p_d = nc.dram_tensor("w0p", [128, 18 * 512], F16, kind="ExternalInput").ap()
    whp_d = nc.dram_tensor("whp", [128, 28 * 512], F16, kind="ExternalInput").ap()
    swp_d = nc.dram_tensor("swp", [128, 16], F16, kind="ExternalInput").ap()
    ident_d = nc.dram_tensor("ident", [128, 128], FP32R, kind="ExternalInput").ap()
    sel128_d = nc.dram_tensor("sel128", [128, 8 * 128], FP32R, kind="ExternalInput").ap()
    zeros_d = nc.dram_tensor("zeros512", [128, BLK], FP32R, kind="ExternalInput").ap()
    gaussT_d = nc.dram_tensor("gaussT", [3, 128], FP32R, kind="ExternalInput").ap()
    woutp_d = nc.dram_tensor("woutp", [128, 16], F16, kind="ExternalInput").ap()
    outT_d = nc.dram_tensor("outT", [4, s_core], FP32, kind="ExternalOutput").ap()

    def r(ap):
        return ap

    with tile.TileContext(nc) as tc:
        with (
            tc.tile_pool(name="const", bufs=1) as constp,
            tc.tile_pool(name="weights", bufs=1) as weightp,
            tc.tile_pool(name="inblk", bufs=2) as inp_pool,
            tc.tile_pool(name="acts", bufs=2) as actp,
            tc.tile_pool(name="scratch", bufs=2) as scr,
            tc.tile_pool(name="ps_t", bufs=1, space="PSUM") as ps_t,
            tc.tile_pool(name="ps_misc", bufs=2, space="PSUM") as ps_misc,
            tc.tile_pool(name="ps_gi", bufs=2, space="PSUM") as ps_gi,
        ):
            # ---- small constants first (so they beat the weight DMAs) ----
            ident_dma = constp.tile([128, 128], FP32R, name="ident_dma")
            nc.sync.dma_start(out=ident_dma, in_=ident_d)
            # DVE-gate the identity so PE transposes only ever wait on DVE
            ident_sb = constp.tile([128, 128], FP32R, name="ident_sb")
            nc.vector.tensor_copy(ident_sb, ident_dma)
            gaussT_sb = constp.tile([3, 128], FP32R)
            nc.sync.dma_start(out=gaussT_sb, in_=gaussT_d)
            sel128_sb = constp.tile([128, 8, 128], FP32R)
            nc.sync.dma_start(
                out=sel128_sb, in_=sel128_d.rearrange("p (m f) -> p m f", m=8))
            # persistent zero-padded corner-weight tile: rows 0-7 are
            # rewritten per block, rows 8-127 stay zero (selector weights
            # there are zero too, but 0*garbage could be NaN)
            w8tp = constp.tile([128, BLK], FP32R, name="w8tp")
            nc.sync.dma_start(out=w8tp, in_=zeros_d)
            wout_sb = weightp.tile([128, 4, 4], F16)
            nc.sync.dma_start(out=wout_sb, in_=woutp_d.rearrange("p (c f) -> p c f", c=4))
            sw_sb = weightp.tile([128, 16], F16)
            nc.sync.dma_start(out=sw_sb, in_=swp_d)

            inp_r = inp_d.rearrange("(b sc p) f -> b p sc f", sc=4, p=128)

            def load_start(b):
                """Issue the input DMA for block b (returns the landing tile)."""
                inb0 = inp_pool.tile([128, 4, 259], FP32, tag="inb0", name="inb0")
                nc.sync.dma_start(out=inb0, in_=inp_r[b])
                return inb0

            # prefetch the first two input blocks before the weight DMAs
            inb0_tiles = {0: load_start(0), 1: load_start(1)}

            # ---- weights, in consumption order ----
            w0_sb = weightp.tile([128, 18, 512], F16)
            w0r = w0p_d.rearrange("p (c f) -> p c f", c=18)
            for ch in range(3):
                nc.sync.dma_start(
                    out=w0_sb[:, ch * 6:(ch + 1) * 6, :], in_=w0r[:, ch * 6:(ch + 1) * 6, :])
            wh_sb = weightp.tile([128, 28, 512], F16)
            whr = whp_d.rearrange("p (c f) -> p c f", c=28)
            for ch in range(4):
                nc.sync.dma_start(
                    out=wh_sb[:, ch * 7:(ch + 1) * 7, :], in_=whr[:, ch * 7:(ch + 1) * 7, :])

            def preamble(b):
                """Emit block b's input-side prep. Called during block b-1's
                hidden phase (or standalone for b == 0): DVE corner-weight
                chain, PE transposes (fp32r, 1.5 c/r), Fourier angle + range
                reduction, and the sin/cos feature tiles."""
                inb = inp_pool.tile([128, 4, 259], FP32R, tag="inb", name="inb")
                nc.vector.tensor_copy(inb, inb0_tiles.pop(b))

                # corner weights in sample layout (DVE only; needs just inb)
                f3 = scr.tile([128, 4, 3], FP32R, tag="f3")
                nc.vector.tensor_scalar(
                    out=f3, in0=inb[:, :, 256:259],
                    scalar1=0.5, scalar2=0.5, op0=ALU.mult, op1=ALU.add,
                )
                om3 = scr.tile([128, 4, 3], FP32R, tag="om3")
                nc.vector.tensor_scalar(
                    out=om3, in0=f3, scalar1=1.0, scalar2=-1.0,
                    op0=ALU.subtract, op1=ALU.mult,
                )
                wxy = scr.tile([128, 4, 4], FP32R, tag="wxy")
                w8s = scr.tile([128, 4, 8], FP32R, tag="w8s")
                for di in (0, 1):
                    xs = (f3 if di else om3)[:, :, 0:1]
                    for dj in (0, 1):
                        ys = (f3 if dj else om3)[:, :, 1:2]
                        nc.vector.tensor_tensor(
                            out=wxy[:, :, di * 2 + dj:di * 2 + dj + 1],
                            in0=xs, in1=ys, op=ALU.mult,
                        )
                for m in range(8):
                    di, dj, dk = (m >> 2) & 1, (m >> 1) & 1, m & 1
                    zsl = (f3 if dk else om3)[:, :, 2:3]
                    nc.vector.tensor_tensor(
                        out=w8s[:, :, m:m + 1],
                        in0=wxy[:, :, di * 2 + dj:di * 2 + dj + 1],
                        in1=zsl, op=ALU.mult,
                    )

                # transposes to [feature, sample] (fp32r: 1.5 cycles/row)
                latT = scr.tile([128, 2, BLK], FP32R, tag="latT", bufs=2)
                xyzT = scr.tile([3, BLK], FP32R, tag="xyzT", bufs=2)
                for sc in range(4):
                    for fc in range(2):
                        tp = ps_misc.tile([128, 128], FP32R, tag="mt")
                        nc.tensor.transpose(
                            tp, inb[:, sc, fc * 128:(fc + 1) * 128], ident_sb
                        )
                        nc.vector.tensor_copy(latT[:, fc, sc * 128:(sc + 1) * 128], tp)
                    tp3 = ps_misc.tile([3, 128], FP32R, tag="mt")
                    nc.tensor.transpose(tp3, inb[:, sc, 256:259], ident_sb)
                    nc.vector.tensor_copy(xyzT[:, sc * 128:(sc + 1) * 128], tp3)
                    tp8 = ps_misc.tile([8, 128], FP32R, tag="mt")
                    nc.tensor.transpose(tp8, w8s[:, sc, :], ident_sb)
                    nc.vector.tensor_copy(w8tp[0:8, sc * 128:(sc + 1) * 128], tp8)

                # fourier angle, range-reduced to [-0.5, 0.5]
                angp = ps_misc.tile([128, BLK], FP32, tag="mt")
                nc.tensor.matmul(angp, r(gaussT_sb), r(xyzT), start=True, stop=True)
                ang_sb = scr.tile([128, BLK], FP32, tag="rr", bufs=3, name="ang_sb")
                nc.vector.tensor_copy(ang_sb, angp)
                zs_r = scr.tile([128, BLK], FP32, tag="rr", bufs=3, name="zs_r")
                nc.vector.tensor_scalar(
                    out=zs_r, in0=ang_sb, scalar1=MAGIC, scalar2=MAGIC,
                    op0=ALU.add, op1=ALU.subtract,
                )
                zs = scr.tile([128, BLK], FP32, tag="zs", bufs=1)
                nc.vector.tensor_sub(zs, ang_sb, zs_r)
                a25 = scr.tile([128, BLK], FP32, tag="a25", bufs=1)
                nc.vector.tensor_scalar_add(out=a25, in0=ang_sb, scalar1=0.25)
                zc_r = scr.tile([128, BLK], FP32, tag="rr", bufs=3, name="zc_r")
                nc.vector.tensor_scalar(
                    out=zc_r, in0=a25, scalar1=MAGIC, scalar2=MAGIC,
                    op0=ALU.add, op1=ALU.subtract,
                )
                zc = scr.tile([128, BLK], FP32, tag="zc", bufs=1)
                nc.vector.tensor_sub(zc, a25, zc_r)
                ffs = scr.tile([128, BLK], F16, tag="ff", bufs=2, name="ffs")
                nc.scalar.activation(out=ffs, in_=zs, func=AF.Sin, scale=TWO_PI)
                ffc = scr.tile([128, BLK], F16, tag="ff", bufs=2, name="ffc")
                nc.scalar.activation(out=ffc, in_=zc, func=AF.Sin, scale=TWO_PI)

                # pre-build the first two broadcast groups of this block's
                # layer 0 so its PE stream starts with zero DVE latency
                uchq = []
                for m in range(2):
                    bc = ps_misc.tile([128, BLK], FP32, tag="mt")
                    nc.tensor.matmul(
                        bc, r(sel128_sb[:, m, :]), r(w8tp), start=True, stop=True
                    )
                    for kc in range(2):
                        uch = scr.tile([128, BLK], F16, tag="uch", bufs=8)
                        nc.vector.tensor_tensor(
                            out=uch, in0=latT[:, kc, :], in1=bc, op=ALU.mult
                        )
                        uchq.append(uch)
                return latT, ffs, ffc, uchq

            def make_finalize(yp, gp, b):
                """Deferred block finalize: rg = 1/sqrt(gp) via the fp32
                bit-trick seed + 2 Newton steps (all DVE; keeps the ACT
                engine on a single activation table for the whole kernel),
                then yv = 255 * yp * rg, tanh on row 0, and the output DMA.
                Emitted a few uch-products into the NEXT block's layer 0 so
                it never head-of-line-blocks that block's DVE feed."""
                def fin():
                    sd = scr.tile([4, BLK], FP32, tag="nr_sd", bufs=1)
                    t1 = scr.tile([4, BLK], FP32, tag="nr_t1", bufs=1)
                    t2 = scr.tile([4, BLK], FP32, tag="nr_t2", bufs=1)
                    y1 = scr.tile([4, BLK], FP32, tag="nr_y1", bufs=1)
                    yv = scr.tile([4, BLK], FP32, tag="yv", bufs=1)
                    # seed bits = RSQRT_SEED - (bits(gp) >> 1)
                    nc.vector.tensor_scalar(
                        out=t1[0:4, :].bitcast(I32), in0=gp[0:4, :].bitcast(I32),
                        scalar1=1, scalar2=None, op0=ALU.logical_shift_right,
                    )
                    nc.vector.tensor_scalar(
                        out=sd[0:4, :].bitcast(I32), in0=t1[0:4, :].bitcast(I32),
                        scalar1=RSQRT_SEED, scalar2=-1,
                        op0=ALU.subtract, op1=ALU.mult,
                    )
                    # Newton 1: y1 = sd * (1.5 - 0.5 * gp * sd^2)
                    nc.vector.tensor_tensor(out=t1, in0=sd, in1=sd, op=ALU.mult)
                    nc.vector.scalar_tensor_tensor(
                        out=t2, in0=t1, scalar=-0.5, in1=gp[0:4, :],
                        op0=ALU.mult, op1=ALU.mult,
                    )
                    nc.vector.tensor_scalar_add(out=t2, in0=t2, scalar1=1.5)
                    nc.vector.tensor_tensor(out=y1, in0=sd, in1=t2, op=ALU.mult)
                    # Newton 2: rg = y1 * (1.5 - 0.5 * gp * y1^2)
                    nc.vector.tensor_tensor(out=t1, in0=y1, in1=y1, op=ALU.mult)
                    nc.vector.scalar_tensor_tensor(
                        out=t2, in0=t1, scalar=-0.5, in1=gp[0:4, :],
                        op0=ALU.mult, op1=ALU.mult,
                    )
                    nc.vector.tensor_scalar_add(out=t2, in0=t2, scalar1=1.5)
                    nc.vector.tensor_tensor(out=t1, in0=y1, in1=t2, op=ALU.mult)
                    # yv = (yp * 255) * rg ; out row0 = tanh(yv0 / 255)
                    nc.vector.scalar_tensor_tensor(
                        out=yv, in0=yp, scalar=255.0, in1=t1,
                        op0=ALU.mult, op1=ALU.mult,
                    )
                    nc.scalar.activation(
                        out=yv[0:1, :], in_=yv[0:1, :], func=AF.Tanh,
                        scale=1.0 / 255.0,
                    )
                    nc.sync.dma_start(
                        out=outT_d[:, b * BLK:(b + 1) * BLK], in_=yv)
                return fin

            pre_next = preamble(0)
            fin_prev = None

            for b in range(n_blocks):
                latT, ffs, ffc, uchq = pre_next
                if b + 2 < n_blocks:
                    inb0_tiles[b + 2] = load_start(b + 2)

                # ---- layer 0: build u chunks incrementally + matmul ----
                psums = [ps_t.tile([128, BLK], FP32, tag=f"pt{mc}", name=f"pt{mc}")
                         for mc in range(4)]
                uch_i = 0

                def l0_accum(u_ap, last=False):
                    nonlocal uch_i
                    for mc in range(4):
                        nc.tensor.matmul(
                            psums[mc],
                            r(w0_sb[:, uch_i, mc * 128:(mc + 1) * 128]),
                            r(u_ap),
                            start=(uch_i == 0), stop=last,
                        )
                    uch_i += 1

                for m in range(8):
                    if m < 6:
                        bc = ps_misc.tile([128, BLK], FP32, tag="mt")
                        nc.tensor.matmul(
                            bc, r(sel128_sb[:, m + 2, :]), r(w8tp),
                            start=True, stop=True,
                        )
                        for kc in range(2):
                            uch = scr.tile([128, BLK], F16, tag="uch", bufs=8)
                            nc.vector.tensor_tensor(
                                out=uch, in0=latT[:, kc, :], in1=bc, op=ALU.mult
                            )
                            uchq.append(uch)
                    l0_accum(uchq[2 * m])
                    l0_accum(uchq[2 * m + 1])
                l0_accum(ffs)
                l0_accum(ffc, last=True)

                # ---- hidden LN+relu layers; block b+1's preamble is emitted
                # after layer 2 so every engine stream stays deep ----
                x_cur = None
                sq = None
                for j in range(N_LAYERS):
                    if j > 0:
                        psums = [
                            ps_t.tile([128, BLK], FP32, tag=f"pt{mc}", name=f"pt{mc}")
                            for mc in range(4)
                        ]
                        for mc in range(4):
                            for kc in range(4):
                                nc.tensor.matmul(
                                    psums[mc],
                                    r(wh_sb[:, (j - 1) * 4 + kc,
                                            mc * 128:(mc + 1) * 128]),
                                    r(x_cur[:, kc, :]),
                                    start=(kc == 0), stop=(kc == 3),
                                )
                    x_next = actp.tile([128, 4, BLK], F16, tag="xn")
                    for mc in range(4):
                        nc.scalar.activation(
                            out=x_next[:, mc, :], in_=psums[mc], func=AF.Relu
                        )
                    if j == 7:
                        sq = scr.tile([128, 4, BLK], F16, tag="sq", bufs=1)
                        for mc in range(4):
                            nc.scalar.activation(
                                out=sq[:, mc, :], in_=psums[mc], func=AF.Square
                            )
                    x_cur = x_next
                    if j == 1 and b + 1 < n_blocks:
                        pre_next = preamble(b + 1)
                    if j == 5 and fin_prev is not None:
                        fin_prev()
                        fin_prev = None

                # ---- output layer, then deferred-LN stats (sq is ready by
                # the time the wout matmuls finish) ----
                yp = ps_gi.tile([4, BLK], FP32, tag="gp", name="yp")
                for kc in range(4):
                    nc.tensor.matmul(
                        yp, r(wout_sb[:, kc, :]), r(x_cur[:, kc, :]),
                        start=(kc == 0), stop=(kc == 3),
                    )
                gp = ps_gi.tile([4, BLK], FP32, tag="gp", name="gp7")
                for mc in range(4):
                    nc.tensor.matmul(
                        gp, r(sw_sb[:, mc * 4:(mc + 1) * 4]),
                        r(sq[:, mc, :]), start=(mc == 0), stop=(mc == 3),
                    )
                fin_prev = make_finalize(yp, gp, b)

            fin_prev()

    nc.compile()
    return nc


def kernel(**inputs):
    if _general_case_needed(inputs):
        return _numpy_fallback(inputs)

    from concourse.bass_utils import run_bass_kernel_spmd

    pre = _precompute(inputs)
    inp = np.ascontiguousarray(np.asarray(inputs["input"], np.float32))

    if "nc" not in _NC_CACHE:
        _NC_CACHE["nc"] = _build_bass()
    nc = _NC_CACHE["nc"]

    in_maps = [
        {
            "inp": np.ascontiguousarray(inp[c * S_CORE:(c + 1) * S_CORE]),
            "w0p": pre["w0p"], "whp": pre["whp"], "swp": pre["swp"],
            "ident": pre["ident"], "gaussT": pre["gaussT"],
            "sel128": pre["sel128"], "woutp": pre["woutp"],
            "zeros512": np.zeros((128, BLK), np.float32),
        }
        for c in range(N_CORES)
    ]

    res = run_bass_kernel_spmd(
        nc, in_maps, core_ids=list(range(N_CORES)),
        trace=bool(int(os.environ.get("KERNEL_TRACE", "0"))),
    )
    kernel.last_results = res
    outs = [res.results[c]["outT"] for c in range(N_CORES)]
    return np.ascontiguousarray(
        np.concatenate([o.T for o in outs], axis=0).astype(np.float32)
    )
